# revision 1
# baseline (speedup 1.0000x reference)
"""DGAD net (vq_codebook) kernel v3 for 8x Trainium2 NeuronCores.

Contract: kernel(**inputs) takes FULL unsharded inputs, returns FULL [4,1]
fp32 output. Batch (128) sharded 16/core; weights replicated; final
all-reduce (sum/128) on host during unshard.

v3 vs v2 (89us):
  - All weight DMAs host-pre-permuted to plain contiguous 2D copies: v2's
    `rearrange` DMAs generated 1KB-granular descriptor storms that serialized
    the Sync engine (15.4us for ow1 alone) and starved the rings.
  - HW pooling split across three engines (each ~128-elem/cycle class):
      DVE:  x_mid ch 0-127   ([c,b,hw] layout, tensor_reduce)  + x_deep b0-7
      ACT:  x_mid ch 128-255 ([c,b,hw] layout, Copy+accum_out)
      PE:   x_mid ch 256-511 ([hw,b*c] layout, fp8 DoubleRow stationary
            ingest: lhsT [128,2,128] packs 2 hw-tiles/LDWEIGHTS) + x_deep
            b8-15 (two samples packed on 98 partitions, 2-col ones rhs).
  - PE warm-up spin (dummy matmuls on catid) so the HAM clock gate sits at
    2.4GHz before the real work arrives.
  - PE tail-first emission: pools -> ow1 -> M -> sw2/sw3 -> rest, so the
    critical texture path clears the PE queue first.
"""

import numpy as np
import ml_dtypes

N_CORES = 8
B = 128
BC = B // N_CORES  # 16 samples per core

BF = ml_dtypes.bfloat16
F8 = ml_dtypes.float8_e4m3
WSCALE = 256.0  # fp8 weights stored *256; 1/256 folded into consumer scales

_CACHE = {}


def _build_program():
    import concourse.bass as bass  # noqa: F401
    import concourse.mybir as mybir
    import concourse.tile as tile
    from concourse import bacc
    from contextlib import ExitStack

    dt = mybir.dt
    AF = mybir.ActivationFunctionType
    ALU = mybir.AluOpType
    AX = mybir.AxisListType
    f32, bf16, f8 = dt.float32, dt.bfloat16, dt.float8e4
    INV = 1.0 / WSCALE
    DR = mybir.MatmulPerfMode.DoubleRow

    from concourse.hw_specs import get_activation_tables
    _act_set_id = list(get_activation_tables("gen3")).index("natural_log_exp_and_others")

    nc = bacc.Bacc("TRN2", target_bir_lowering=False, debug=False,
                   enable_asserts=True, num_devices=N_CORES)

    def din(name, shape, d):
        return nc.dram_tensor(name, shape, d, kind="ExternalInput").ap()

    xmV_d = din("xmV", [128, 16, 784], f8)   # ch 0-127, [c,b,hw] (DVE)
    xmA_d = din("xmA", [128, 16, 784], f8)   # ch 128-255, [c,b,hw] (ACT)
    xmP_d = din("xmP", [784, 4096], f8)      # ch 256-511, [hw, b*256+(c-256)] (PE)
    xdV_d = din("xdV", [128, 8, 16, 49], f8)  # b0-7, [d%128, b, d//128, hw] (DVE)
    xdP_d = din("xdP", [98, 8192], f8)       # b8-15 2-packed, [hw(+49*par), j*2048+d] (PE)
    ow1T_d = din("ow1T", [128, 16, 1024], f8)  # pre-permuted (k p) o -> p k o
    MT_d = din("MT", [128, 4, 1024], f8)       # (wsh.T @ sw1.T)*256, pre-permuted
    ow2T_d = din("ow2T", [128, 8, 512], f8)
    sw2T_d = din("sw2T", [128, 8, 512], f8)
    ow3T_d = din("ow3T", [128, 4, 64], f8)
    sw3T_d = din("sw3T", [128, 4, 64], f8)
    tw1T_d = din("tw1T", [128, 64], f32)
    tw2T_d = din("tw2T", [64, 64], f32)
    cw1T_d = din("cw1T", [128, 64], f32)
    cw2T_d = din("cw2T", [64, 64], f32)
    qw1T_d = din("qw1T", [64, 64], f32)
    qw2T_d = din("qw2T", [64, 64], f32)
    protoT_d = din("protoT", [64, 4], f32)
    proto_pad_d = din("proto_pad", [4, 128], f32)
    center_pad_d = din("center_pad", [1, 128], f32)
    center_col_d = din("center_col", [64, 1], f32)
    catid_d = din("catid", [64, 128], f32)
    id16_d = din("id16", [16, 16], f32)
    ones2_d = din("ones2", [98, 2], bf16)     # [:49]=[1,0], [49:]=[0,1]
    ones_f8_d = din("ones_f8", [128, 2, 1], f8)
    out_d = nc.dram_tensor("out", [1, 4], f32, kind="ExternalOutput").ap()

    with tile.TileContext(nc) as tc, ExitStack() as ctx:
        wp = ctx.enter_context(tc.tile_pool(name="wp", bufs=1))
        xp = ctx.enter_context(tc.tile_pool(name="xp", bufs=1))
        ap = ctx.enter_context(tc.tile_pool(name="ap", bufs=1))
        pp = ctx.enter_context(tc.tile_pool(name="pp", bufs=3, space="PSUM"))
        pt = ctx.enter_context(tc.tile_pool(name="pt", bufs=2, space="PSUM"))
        pmp = ctx.enter_context(tc.tile_pool(name="pmp", bufs=1, space="PSUM"))
        pdp = ctx.enter_context(tc.tile_pool(name="pdp", bufs=1, space="PSUM"))

        # ---------- ACT table preload (Lrelu set loads during startup) ----------
        scr = ap.tile([1, 1], f32, tag="scr")
        scr2 = ap.tile([1, 1], f32, tag="scr2")
        nc.gpsimd.memset(scr[:], 1.0)
        nc.scalar.activation(scr2[:], scr[:], AF.Lrelu, alpha=0.01)

        # ---------- tiles ----------
        ow1_t = wp.tile([128, 16, 1024], f8, tag="ow1")
        MT_t = wp.tile([128, 4, 1024], f8, tag="MT")
        ow2_t = wp.tile([128, 8, 512], f8, tag="ow2")
        sw2_t = wp.tile([128, 8, 512], f8, tag="sw2")
        ow3_t = wp.tile([128, 4, 64], f8, tag="ow3")
        sw3_t = wp.tile([128, 4, 64], f8, tag="sw3")
        tw1_t = wp.tile([128, 64], f32, tag="tw1")
        tw2_t = wp.tile([64, 64], f32, tag="tw2")
        cw1_t = wp.tile([128, 64], f32, tag="cw1")
        cw2_t = wp.tile([64, 64], f32, tag="cw2")
        qw1_t = wp.tile([64, 64], f32, tag="qw1")
        qw2_t = wp.tile([64, 64], f32, tag="qw2")
        protoT_t = wp.tile([64, 4], f32, tag="protoT")
        proto_pad_t = wp.tile([4, 128], f32, tag="proto_pad")
        center_pad_t = wp.tile([1, 128], f32, tag="center_pad")
        center_col_t = wp.tile([64, 1], f32, tag="center_col")
        catid_t = wp.tile([64, 128], f32, tag="catid")
        id16_t = wp.tile([16, 16], f32, tag="id16")
        ones2_t = wp.tile([98, 2], bf16, tag="ones2")
        ones_f8_t = wp.tile([128, 2, 1], f8, tag="ones_f8")

        xmV_t = xp.tile([128, 16, 784], f8, tag="xmV")
        xmA_t = xp.tile([128, 16, 784], f8, tag="xmA")
        xmP_t = xp.tile([128, 7, 4096], f8, tag="xmP")
        xdV_t = xp.tile([128, 8, 16, 49], f8, tag="xdV")
        xdP_t = xp.tile([98, 8192], f8, tag="xdP")
        scratch = ap.tile([128, 784], bf16, tag="scratch")

        pool_m = pmp.tile([128, 32], f32, tag="pool_m")      # [c-in-g, b*2+(g-2)]
        pool_d = pdp.tile([128, 4, 16, 2], f32, tag="pool_d")  # [d-in-dc, j, dc, side]
        pooled_v = ap.tile([128, 16], f32, tag="pooled_v")
        pooled_a = ap.tile([128, 16], f32, tag="pooled_a")
        pooled_dv = ap.tile([128, 8, 16], f32, tag="pooled_dv")

        # ---------- DMAs ----------
        for t_, d_ in ((ones2_t, ones2_d), (ones_f8_t, ones_f8_d),
                       (tw1_t, tw1T_d), (tw2_t, tw2T_d),
                       (cw1_t, cw1T_d), (cw2_t, cw2T_d), (qw1_t, qw1T_d),
                       (qw2_t, qw2T_d), (protoT_t, protoT_d),
                       (proto_pad_t, proto_pad_d), (center_pad_t, center_pad_d),
                       (center_col_t, center_col_d), (catid_t, catid_d),
                       (id16_t, id16_d)):
            nc.sync.dma_start(out=t_[:], in_=d_)
        nc.sync.dma_start(out=xdV_t[:, 0:4, :, :], in_=xdV_d[:, 0:4, :, :])
        nc.sync.dma_start(out=xdV_t[:, 4:8, :, :], in_=xdV_d[:, 4:8, :, :])
        nc.sync.dma_start(out=xdP_t[:], in_=xdP_d)
        nc.sync.dma_start(out=xmA_t[:], in_=xmA_d)
        for q in range(4):
            nc.sync.dma_start(out=xmV_t[:, 4 * q:4 * q + 4, :],
                              in_=xmV_d[:, 4 * q:4 * q + 4, :])
        for h in range(6):
            nc.sync.dma_start(out=xmP_t[:, h, :], in_=xmP_d[128 * h:128 * h + 128, :])
        nc.sync.dma_start(out=xmP_t[0:16, 6, :], in_=xmP_d[768:784, :])
        nc.sync.dma_start(out=ow1_t[:], in_=ow1T_d)
        nc.sync.dma_start(out=MT_t[:], in_=MT_d)
        nc.sync.dma_start(out=sw2_t[:], in_=sw2T_d)
        nc.sync.dma_start(out=sw3_t[:], in_=sw3T_d)
        nc.sync.dma_start(out=ow2_t[:], in_=ow2T_d)
        nc.sync.dma_start(out=ow3_t[:], in_=ow3T_d)

        # ---------- small consts (gpsimd/DVE, during startup) ----------
        ones64 = ap.tile([64, 1], f32, tag="ones64")
        nc.gpsimd.memset(ones64[:], 1.0)
        ones16 = ap.tile([16, 1], f32, tag="ones16")
        nc.gpsimd.memset(ones16[:], 1.0)
        ones1x16 = ap.tile([1, 16], f32, tag="ones1x16")
        nc.gpsimd.memset(ones1x16[:], 1.0)
        neg_center_pad = ap.tile([1, 128], f32, tag="ncp")
        nc.vector.tensor_scalar(neg_center_pad[:], center_pad_t[:], -1.0, None, op0=ALU.mult)
        neg_ppad = ap.tile([4, 128], f32, tag="npp")
        nc.vector.tensor_scalar(neg_ppad[:], proto_pad_t[:], -1.0, None, op0=ALU.mult)
        rhs_sim = ap.tile([65, 4], f32, tag="rhs_sim")
        nc.vector.tensor_scalar(rhs_sim[0:64, :], protoT_t[:], -2.0, None, op0=ALU.mult)
        nc.gpsimd.memset(rhs_sim[64:65, :], 1.0)
        pT2 = ap.tile([64, 4], f32, tag="pT2")
        nc.vector.tensor_tensor(pT2[:], protoT_t[:], protoT_t[:], op=ALU.mult)

        # ---------- PE warm-up spin (HAM clock gate) ----------
        warm_ps = pt.tile([128, 16], f32, tag="tail")
        for _ in range(48):
            nc.tensor.matmul(warm_ps[:, 0:1], catid_t[:], ones64[:],
                             start=True, stop=True)

        pn_ps = pt.tile([128, 16], f32, tag="tail")
        nc.tensor.matmul(pn_ps[0:1, 0:4], ones64[:], pT2[:], start=True, stop=True)
        pnorm = ap.tile([1, 4], f32, tag="pnorm")
        nc.scalar.copy(pnorm[:], pn_ps[0:1, 0:4])

        # ---------- PE: x_deep b8-15 pool (2-sample packed) ----------
        for t in range(64):  # t = j*16 + dc
            nc.tensor.matmul(pool_d[:, t // 16, t % 16, :],
                             xdP_t[:, 128 * t:128 * t + 128],
                             ones2_t[:], start=True, stop=True)

        # ---------- PE: x_mid ch 256-511 pool (DoubleRow pairs of hw-tiles) ----
        for hp in (0, 2, 4):
            for t in range(32):
                nc.tensor.matmul(pool_m[:, t:t + 1],
                                 xmP_t[:, hp:hp + 2, 128 * t:128 * t + 128],
                                 ones_f8_t[:, :, :], perf_mode=DR,
                                 start=(hp == 0), stop=False)
        for t in range(32):
            nc.tensor.matmul(pool_m[:, t:t + 1],
                             xmP_t[0:16, 6, 128 * t:128 * t + 128],
                             ones_f8_t[0:16, 0, :], start=False, stop=True)

        # ---------- DVE: x_deep b0-7 + x_mid ch 0-127 pools ----------
        for hf in range(2):
            nc.vector.reduce_sum(pooled_dv[:, 4 * hf:4 * hf + 4, :],
                                 xdV_t[:, 4 * hf:4 * hf + 4, :, :], axis=AX.X)
        for q in range(4):
            nc.vector.reduce_sum(pooled_v[:, 4 * q:4 * q + 4],
                                 xmV_t[:, 4 * q:4 * q + 4, :], axis=AX.X)

        # ---------- ACT: x_mid ch 128-255 pool ----------
        for b in range(16):
            nc.scalar.activation(scratch[:], xmA_t[:, b, :], AF.Copy,
                                 accum_out=pooled_a[:, b:b + 1])

        # ---------- evacs ----------
        # xdb [128, 8, 2, 16] = [d-in-dc, j, side, dc]; b = 2j+side (0-7 DVE, 8-15 PE)
        xdb = ap.tile([128, 8, 2, 16], bf16, tag="xdb")
        nc.scalar.mul(xdb[:, 0:4, :, :], pooled_dv[:], INV / 49.0)
        for s in range(2):
            nc.scalar.mul(xdb[:, 4:8, s, :], pool_d[:, :, :, s], INV / 49.0)
        # xmb [128, 16, 4] = [c-in-g, b, g]
        xmb = ap.tile([128, 16, 4], bf16, tag="xmb")
        nc.scalar.mul(xmb[:, :, 0], pooled_v[:], INV / 784.0)
        nc.scalar.mul(xmb[:, :, 1], pooled_a[:], INV / 784.0)
        nc.scalar.mul(xmb[:, :, 2:4], pool_m[:], INV / 784.0)

        # generic chain layer; act_scale folds the fp8 weight prescale
        def layer(w_t, n_k, n_m, m_sz, rhs_fn, dst_fn, act_scale, act=True):
            for m in range(n_m):
                ps = pp.tile([128, 16], f32, tag="mm")
                for k in range(n_k):
                    nc.tensor.matmul(ps[:m_sz, :], w_t[:, k, m * m_sz:(m + 1) * m_sz],
                                     rhs_fn(k), start=(k == 0), stop=(k == n_k - 1))
                if act:
                    nc.scalar.activation(dst_fn(m), ps[:m_sz, :], AF.Lrelu,
                                         scale=act_scale, alpha=0.01)
                else:
                    nc.scalar.mul(dst_fn(m), ps[:m_sz, :], act_scale)

        # ---------- origin layer 1 (PE next; its weights land before MT) -------
        y1o = ap.tile([128, 8, 16], bf16, tag="y1o")
        layer(ow1_t, 16, 8, 128, lambda k: xdb[:, :, :, k], lambda m: y1o[:, m, :], 1.0)

        # ---------- shallow chain (conv folded into M) ----------
        y1s = ap.tile([128, 8, 16], bf16, tag="y1s")
        layer(MT_t, 4, 8, 128, lambda k: xmb[:, :, k], lambda m: y1s[:, m, :], 1.0)
        y2s = ap.tile([128, 4, 16], bf16, tag="y2s")
        layer(sw2_t, 8, 4, 128, lambda k: y1s[:, k, :], lambda m: y2s[:, m, :], INV)
        shallow = ap.tile([64, 16], f32, tag="shallow")
        layer(sw3_t, 4, 1, 64, lambda k: y2s[:, k, :], lambda m: shallow[:], INV)

        def small_mlp(wa, wb, rhs, dst, tagn):
            psa = pt.tile([128, 16], f32, tag="tail")
            nc.tensor.matmul(psa[:64, :], wa[:, 0:64], rhs, start=True, stop=True)
            mid = ap.tile([64, 16], f32, tag="mid_" + tagn)
            nc.scalar.activation(mid[:], psa[:64, :], AF.Lrelu, alpha=0.01)
            psb = pt.tile([128, 16], f32, tag="tail")
            nc.tensor.matmul(psb[:64, :], wb[:, 0:64], mid[:], start=True, stop=True)
            return nc.scalar.activation(dst, psb[:64, :], AF.Lrelu, alpha=0.01)

        def svdd(feat, dst_sb, tagn):
            d_ = ap.tile([64, 16], f32, tag="d_" + tagn)
            nc.vector.tensor_scalar(d_[:], feat, center_col_t[:, 0:1], None, op0=ALU.subtract)
            sq = ap.tile([64, 16], f32, tag="sq_" + tagn)
            nc.vector.tensor_tensor(sq[:], d_[:], d_[:], op=ALU.mult)
            psv = pt.tile([128, 16], f32, tag="tail")
            nc.tensor.matmul(psv[0:1, :], ones64[:], sq[:], start=True, stop=True)
            nc.scalar.copy(dst_sb, psv[0:1, :])

        # ---------- texture ----------
        cat1_ps = pt.tile([128, 16], f32, tag="tail")
        nc.tensor.matmul(cat1_ps[:], catid_t[:], shallow[:], start=True, stop=False)
        nc.tensor.matmul(cat1_ps[:], neg_center_pad[:], ones1x16[:], start=False, stop=True)
        cat1 = ap.tile([128, 16], f32, tag="cat1")
        nc.scalar.copy(cat1[:], cat1_ps[:])

        t1ps = pt.tile([128, 16], f32, tag="tail")
        nc.tensor.matmul(t1ps[:64, :], tw1_t[:, 0:64], cat1[:], start=True, stop=True)
        t1 = ap.tile([64, 16], f32, tag="t1")
        nc.scalar.activation(t1[:], t1ps[:64, :], AF.Lrelu, alpha=0.01)
        sim_lhs = ap.tile([65, 16], f32, tag="sim_lhs")
        t2ps = pt.tile([128, 16], f32, tag="tail")
        nc.tensor.matmul(t2ps[:64, :], tw2_t[:, 0:64], t1[:], start=True, stop=True)
        nc.scalar.activation(sim_lhs[0:64, :], t2ps[:64, :], AF.Lrelu, alpha=0.01)

        # ---------- origin layers 2-3 + qw chain + origin_svdd ----------
        y2o = ap.tile([128, 4, 16], bf16, tag="y2o")
        layer(ow2_t, 8, 4, 128, lambda k: y1o[:, k, :], lambda m: y2o[:, m, :], INV)
        origin = ap.tile([64, 16], f32, tag="origin")
        layer(ow3_t, 4, 1, 64, lambda k: y2o[:, k, :], lambda m: origin[:], INV)
        qf = ap.tile([64, 16], f32, tag="qf")
        small_mlp(qw1_t, qw2_t, origin[:], qf[:], "q")
        osvdd = ap.tile([1, 16], f32, tag="osvdd")
        svdd(qf[:], osvdd[:], "o")

        # ---------- sim + CE + argmax ----------
        t2 = ap.tile([64, 16], f32, tag="t2")
        nc.vector.tensor_tensor(t2[:], sim_lhs[0:64, :], sim_lhs[0:64, :], op=ALU.mult)
        tsq_ps = pt.tile([128, 16], f32, tag="tail")
        nc.tensor.matmul(tsq_ps[0:1, :], ones64[:], t2[:], start=True, stop=True)
        nc.scalar.copy(sim_lhs[64:65, :], tsq_ps[0:1, :])

        sim_ps = pt.tile([128, 16], f32, tag="tail")
        nc.tensor.matmul(sim_ps[0:16, 0:4], sim_lhs[:], rhs_sim[:], start=True, stop=False)
        nc.tensor.matmul(sim_ps[0:16, 0:4], ones1x16[:], pnorm[:], start=False, stop=True)
        sim_sb = ap.tile([16, 4], f32, tag="sim_sb")
        nc.vector.tensor_copy(sim_sb[:], sim_ps[0:16, 0:4])

        m16 = ap.tile([16, 1], f32, tag="m16")
        nc.vector.reduce_max(m16[:], sim_sb[:], axis=AX.X)
        negm = ap.tile([16, 1], f32, tag="negm")
        nc.vector.reduce_max(negm[:], sim_sb[:], axis=AX.X, negate=True)

        onehotT = ap.tile([16, 4], f32, tag="onehotT")
        nc.vector.tensor_scalar(onehotT[:], sim_sb[:], m16[:, 0:1], None, op0=ALU.is_ge)
        oh_ps = pt.tile([128, 16], f32, tag="tail")
        nc.tensor.transpose(oh_ps[0:4, 0:16], onehotT[:], id16_t[:])
        oh_sb = ap.tile([4, 16], f32, tag="oh_sb")
        nc.vector.tensor_copy(oh_sb[:], oh_ps[0:4, 0:16])

        # ---------- class feat chain ----------
        cat2_ps = pt.tile([128, 16], f32, tag="tail")
        nc.tensor.matmul(cat2_ps[:], catid_t[:], origin[:], start=True, stop=False)
        nc.tensor.matmul(cat2_ps[:], neg_ppad[:], oh_sb[:], start=False, stop=True)
        cat2 = ap.tile([128, 16], f32, tag="cat2")
        nc.scalar.copy(cat2[:], cat2_ps[:])

        cf = ap.tile([64, 16], f32, tag="cf")
        last_lrelu = small_mlp(cw1_t, cw2_t, cat2[:], cf[:], "c")
        csvdd = ap.tile([1, 16], f32, tag="csvdd")
        svdd(cf[:], csvdd[:], "c")

        # ---------- align + partials ----------
        al = ap.tile([1, 16], f32, tag="al")
        nc.vector.tensor_tensor(al[:], osvdd[:], csvdd[:], op=ALU.subtract)
        aln = ap.tile([1, 16], f32, tag="aln")
        nc.vector.tensor_scalar(aln[:], al[:], -1.0, None, op0=ALU.mult)
        nc.vector.tensor_tensor(al[:], al[:], aln[:], op=ALU.max)

        outv = ap.tile([1, 4], f32, tag="outv")
        nc.vector.reduce_sum(outv[0:1, 1:2], osvdd[:], axis=AX.X)
        nc.vector.reduce_sum(outv[0:1, 2:3], csvdd[:], axis=AX.X)
        nc.vector.reduce_sum(outv[0:1, 3:4], al[:], axis=AX.X)

        # ---------- CE last (Exp/Ln share one table set; single switch) ----------
        ldset = mybir.InstLoadActFuncSet(
            name=f"I-{nc.next_id()}", act_func_set_id=_act_set_id, ins=[], outs=[])
        ldset.engine = mybir.EngineType.Activation
        ld_bi = nc.scalar.add_instruction(ldset)
        tile.add_dep_helper(ldset, last_lrelu.ins, sync=False,
                            reason="table load after last Lrelu")
        e_t = ap.tile([16, 4], f32, tag="e_t")
        s16 = ap.tile([16, 1], f32, tag="s16")
        exp_bi = nc.scalar.activation(e_t[:], sim_sb[:], AF.Exp, bias=negm[:, 0:1],
                                      accum_out=s16[:])
        tile.add_dep_helper(exp_bi.ins, ldset, sync=False,
                            reason="Exp after manual table load")
        ce_col = ap.tile([16, 1], f32, tag="ce_col")
        nc.scalar.activation(ce_col[:], s16[:], AF.Ln)
        ce_ps = pt.tile([128, 16], f32, tag="tail")
        nc.tensor.matmul(ce_ps[0:1, 0:1], ce_col[:], ones16[:], start=True, stop=True)
        nc.vector.tensor_copy(outv[0:1, 0:1], ce_ps[0:1, 0:1])
        nc.sync.dma_start(out=out_d[:], in_=outv[:])

    nc.compile()
    return nc


def _host_prep(inputs):
    f = np.float32
    xm8 = np.asarray(inputs["x_mid"], f).reshape(B, 512, 784).astype(F8)
    xd8 = np.asarray(inputs["x_deep"], f).reshape(B, 2048, 49).astype(F8)

    def T(w):
        return np.ascontiguousarray(np.asarray(w, f).T)

    def T8(w):
        return (T(w) * WSCALE).astype(F8)

    def ptile(w, kk):  # [K, O] -> [128, kk, O] with row k*128+p -> [p, k, :]
        K, O = w.shape
        return np.ascontiguousarray(w.reshape(kk, 128, O).transpose(1, 0, 2))

    M = np.asarray(inputs["w_shallow"], f).T @ np.asarray(inputs["sw1"], f).T

    center = np.asarray(inputs["center"], f)
    proto = np.asarray(inputs["proto"], f)
    proto_pad = np.zeros((4, 128), f)
    proto_pad[:, 64:] = proto
    center_pad = np.zeros((1, 128), f)
    center_pad[0, 64:] = center
    catid = np.zeros((64, 128), f)
    catid[np.arange(64), np.arange(64)] = 1
    catid[np.arange(64), 64 + np.arange(64)] = 1
    ones2 = np.zeros((98, 2), dtype=BF)
    ones2[0:49, 0] = 1
    ones2[49:98, 1] = 1

    shared = {
        "ow1T": ptile(T8(inputs["ow1"]), 16),
        "MT": ptile((M * WSCALE).astype(F8), 4),
        "ow2T": ptile(T8(inputs["ow2"]), 8),
        "sw2T": ptile(T8(inputs["sw2"]), 8),
        "ow3T": ptile(T8(inputs["ow3"]), 4),
        "sw3T": ptile(T8(inputs["sw3"]), 4),
        "tw1T": T(inputs["tw1"]),
        "tw2T": T(inputs["tw2"]),
        "cw1T": T(inputs["cw1"]),
        "cw2T": T(inputs["cw2"]),
        "qw1T": T(inputs["qw1"]),
        "qw2T": T(inputs["qw2"]),
        "protoT": T(proto),
        "proto_pad": proto_pad,
        "center_pad": center_pad,
        "center_col": np.ascontiguousarray(center.reshape(64, 1)),
        "catid": catid,
        "id16": np.eye(16, dtype=f),
        "ones2": ones2,
        "ones_f8": np.ones((128, 2, 1), dtype=F8),
    }
    in_maps = []
    for c in range(N_CORES):
        m = dict(shared)
        xc = xm8[c * BC:(c + 1) * BC]          # [16, 512, 784]
        xdc = xd8[c * BC:(c + 1) * BC]         # [16, 2048, 49]
        m["xmV"] = np.ascontiguousarray(xc[:, 0:128].transpose(1, 0, 2))
        m["xmA"] = np.ascontiguousarray(xc[:, 128:256].transpose(1, 0, 2))
        m["xmP"] = np.ascontiguousarray(
            xc[:, 256:512].transpose(2, 0, 1)).reshape(784, 4096)
        # xdV: b0-7, [d%128... actually [p, b, dchunk, hw] with d = dchunk*128+p
        m["xdV"] = np.ascontiguousarray(
            xdc[0:8].reshape(8, 16, 128, 49).transpose(2, 0, 1, 3))
        # xdP: b8-15 packed 2 samples per partition set (even b upper, odd lower)
        hi = xdc[8:16]                          # [8, 2048, 49]
        ev = hi[0::2].transpose(2, 0, 1)        # [49, 4, 2048]
        od = hi[1::2].transpose(2, 0, 1)
        m["xdP"] = np.ascontiguousarray(
            np.concatenate([ev, od], axis=0)).reshape(98, 8192)
        in_maps.append(m)
    return in_maps


def _get_program():
    if "nc" not in _CACHE:
        _CACHE["nc"] = _build_program()
    return _CACHE["nc"]


def _combine(parts):
    tot = np.sum([np.asarray(p, np.float64).ravel() for p in parts], axis=0)
    return (tot / B).astype(np.float32).reshape(4, 1)


def _run(inputs, trace=False):
    from concourse.bass_utils import run_bass_kernel_spmd
    nc = _get_program()
    in_maps = _host_prep(inputs)
    kw = {}
    if trace:
        kw = dict(trace=True, trace_cores=list(range(N_CORES)))
    res = run_bass_kernel_spmd(nc, in_maps, list(range(N_CORES)), **kw)
    out = _combine([res.results[i]["out"] for i in range(N_CORES)])
    return out, res


def kernel(**inputs):
    out, _ = _run(inputs, trace=False)
    return out


def kernel_traced(**inputs):
    """Returns (output, exec_time_ns) using the NTFF profile (max over cores)."""
    out, res = _run(inputs, trace=True)
    return out, res.exec_time_ns



# revision 5
# speedup vs baseline: 1.0643x; 1.0643x over previous
"""DGAD net (vq_codebook) kernel v4 for 8x Trainium2 NeuronCores.

Contract: kernel(**inputs) takes FULL unsharded inputs, returns FULL [4,1]
fp32 output. Batch (128) sharded 16/core; weights replicated; final
all-reduce (sum/128) on host during unshard.

v4 vs v3 (85.4us):
  - No DoubleRow on PE pools (DR is a net loss at free-dim<128: ~120ns/MM
    vs ~40ns; LDWEIGHTS scales with cols, not rows).
  - PE queue reordered: origin chain (ow1/ow2/ow3/qw, x_deep-only deps)
    runs DURING x_mid streaming, interleaved with pool_m chunk batches,
    instead of after all pools.
  - Two HWDGE rings (Sync + ACT) issue DMAs; chunks interleaved so each
    pool engine streams as data arrives. x-inputs chunked 4-8 ways.
  - All lrelu evacs on DVE via scalar_tensor_tensor (0.01*x max x); ACT
    keeps the Exp/Ln table set loaded from t=0 (no mid-tail table switch).
  - Per-layer matmuls accumulate into ONE psum bank region ([128,8,16]),
    single DVE evac, instead of per-m-chunk psum tiles + evacs.
  - Small weights packed into 4 blob DMAs (blob128/blob64/cat_blob/id16).
"""

import numpy as np
import ml_dtypes

N_CORES = 8
B = 128
BC = B // N_CORES  # 16 samples per core

BF = ml_dtypes.bfloat16
F8 = ml_dtypes.float8_e4m3
WSCALE = 256.0  # fp8 weights stored *256; 1/256 folded into consumer scales

_CACHE = {}


def _build_program():
    import concourse.bass as bass  # noqa: F401
    import concourse.mybir as mybir
    import concourse.tile as tile
    from concourse import bacc
    from contextlib import ExitStack

    dt = mybir.dt
    AF = mybir.ActivationFunctionType
    ALU = mybir.AluOpType
    AX = mybir.AxisListType
    f32, bf16, f8 = dt.float32, dt.bfloat16, dt.float8e4
    INV = 1.0 / WSCALE
    INV2 = INV * INV

    from concourse.hw_specs import get_activation_tables
    _act_set_id = list(get_activation_tables("gen3")).index("natural_log_exp_and_others")

    nc = bacc.Bacc("TRN2", target_bir_lowering=False, debug=False,
                   enable_asserts=True, num_devices=N_CORES)

    def din(name, shape, d):
        return nc.dram_tensor(name, shape, d, kind="ExternalInput").ap()

    xmV_d = din("xmV", [128, 16, 784], f8)    # ch 0-127, [c,b,hw] (DVE)
    xmA_d = din("xmA", [128, 16, 784], f8)    # ch 128-255, [c,b,hw] (ACT)
    xmP_d = din("xmP", [784, 4096], f8)       # ch 256-511, [hw,(ct,b,c_lo)] (PE)
    xdV_d = din("xdV", [128, 8, 16, 49], f8)  # b0-7, [d%128, b, d//128, hw] (DVE)
    xdP_d = din("xdP", [98, 8192], f8)        # b8-15 2-packed, [hw(+49*par), j*2048+d] (PE)
    ow1T_d = din("ow1T", [128, 16, 1024], f8)   # (k p) o -> p k o, *256
    MT_d = din("MT", [128, 4, 1024], f8)        # (wsh.T @ sw1.T)*256, pre-permuted
    ow2T_d = din("ow2T", [128, 8, 512], f8)
    sw2T_d = din("sw2T", [128, 8, 512], f8)
    o3s3_d = din("o3s3", [128, 4, 128], f8)     # cols 0:64 ow3T, 64:128 sw3T
    blob128_d = din("blob128", [128, 128], f32)  # cols 0:64 tw1T, 64:128 cw1T
    blob64_d = din("blob64", [64, 261], f32)     # tw2|cw2|qw1|qw2|protoT|center
    cat_blob_d = din("cat_blob", [64, 128], f32)  # catid
    ppad_d = din("ppad", [4, 128], f32)           # cols 64: = -proto
    cpad_d = din("cpad", [1, 128], f32)           # cols 64: = -center
    id16_d = din("id16", [16, 16], f32)
    ones2_d = din("ones2", [98, 2], bf16)       # [:49]=[1,0], [49:]=[0,1]
    onescol_d = din("onescol", [128, 1], bf16)
    out_d = nc.dram_tensor("out", [1, 4], f32, kind="ExternalOutput").ap()

    with tile.TileContext(nc) as tc, ExitStack() as ctx:
        wp = ctx.enter_context(tc.tile_pool(name="wp", bufs=1))
        xp = ctx.enter_context(tc.tile_pool(name="xp", bufs=1))
        ap = ctx.enter_context(tc.tile_pool(name="ap", bufs=1))
        pmp = ctx.enter_context(tc.tile_pool(name="pmp", bufs=1, space="PSUM"))
        pdp = ctx.enter_context(tc.tile_pool(name="pdp", bufs=1, space="PSUM"))
        pbig = ctx.enter_context(tc.tile_pool(name="pbig", bufs=2, space="PSUM"))
        pt = ctx.enter_context(tc.tile_pool(name="pt", bufs=3, space="PSUM"))

        # ---------- weight tiles ----------
        ow1_t = wp.tile([128, 16, 1024], f8, tag="ow1")
        MT_t = wp.tile([128, 4, 1024], f8, tag="MT")
        ow2_t = wp.tile([128, 8, 512], f8, tag="ow2")
        sw2_t = wp.tile([128, 8, 512], f8, tag="sw2")
        o3s3_t = wp.tile([128, 4, 128], f8, tag="o3s3")
        blob128_t = wp.tile([128, 128], f32, tag="blob128")
        blob64_t = wp.tile([64, 261], f32, tag="blob64")
        cat_blob_t = wp.tile([64, 128], f32, tag="cat_blob")
        ppad_t = wp.tile([4, 128], f32, tag="ppad")
        cpad_t = wp.tile([1, 128], f32, tag="cpad")
        id16_t = wp.tile([16, 16], f32, tag="id16")
        ones2_t = wp.tile([98, 2], bf16, tag="ones2")
        onescol_t = wp.tile([128, 1], bf16, tag="onescol")

        xmV_t = xp.tile([128, 16, 784], f8, tag="xmV")
        xmA_t = xp.tile([128, 16, 784], f8, tag="xmA")
        xmP_t = xp.tile([128, 7, 4096], f8, tag="xmP")
        xdV_t = xp.tile([128, 8, 16, 49], f8, tag="xdV")
        xdP_t = xp.tile([98, 8192], f8, tag="xdP")

        # weight slices
        tw1 = blob128_t[:, 0:64]
        cw1 = blob128_t[:, 64:128]
        tw2 = blob64_t[:, 0:64]
        cw2 = blob64_t[:, 64:128]
        qw1 = blob64_t[:, 128:192]
        qw2 = blob64_t[:, 192:256]
        protoT = blob64_t[:, 256:260]
        center_col = blob64_t[:, 260:261]
        catid = cat_blob_t[:]
        neg_ppad = ppad_t[:]
        neg_cpad = cpad_t[:]

        # ---------- DMA issue: ACT ring (13) ----------
        for t_, d_ in ((blob128_t, blob128_d), (blob64_t, blob64_d),
                       (cat_blob_t, cat_blob_d), (ppad_t, ppad_d),
                       (cpad_t, cpad_d), (id16_t, id16_d),
                       (ones2_t, ones2_d), (onescol_t, onescol_d)):
            nc.scalar.dma_start(out=t_[:], in_=d_)
        nc.scalar.dma_start(out=xdV_t[:, 0:4, :, :], in_=xdV_d[:, 0:4, :, :])
        nc.scalar.dma_start(out=xdV_t[:, 4:8, :, :], in_=xdV_d[:, 4:8, :, :])
        nc.scalar.dma_start(out=xdP_t[:], in_=xdP_d)
        nc.scalar.dma_start(out=MT_t[:], in_=MT_d)
        nc.scalar.dma_start(out=o3s3_t[:], in_=o3s3_d)
        nc.scalar.dma_start(out=ow2_t[:], in_=ow2T_d)
        nc.scalar.dma_start(out=sw2_t[:], in_=sw2T_d)

        # ---------- DMA issue: Sync ring (19), interleaved arrival order ----
        def dma_ow1(k):
            nc.sync.dma_start(out=ow1_t[:, 4 * k:4 * k + 4, :],
                              in_=ow1T_d[:, 4 * k:4 * k + 4, :])

        def dma_xa(q):
            nc.sync.dma_start(out=xmA_t[:, 4 * q:4 * q + 4, :],
                              in_=xmA_d[:, 4 * q:4 * q + 4, :])

        def dma_xv(q):
            nc.sync.dma_start(out=xmV_t[:, 4 * q:4 * q + 4, :],
                              in_=xmV_d[:, 4 * q:4 * q + 4, :])

        def dma_xp(h):
            if h < 6:
                nc.sync.dma_start(out=xmP_t[:, h, :],
                                  in_=xmP_d[128 * h:128 * h + 128, :])
            else:
                nc.sync.dma_start(out=xmP_t[0:16, 6, :], in_=xmP_d[768:784, :])

        dma_ow1(0); dma_xa(0); dma_ow1(1); dma_xv(0)
        dma_ow1(2); dma_xa(1); dma_ow1(3); dma_xv(1)
        dma_xp(0); dma_xa(2); dma_xp(1); dma_xv(2)
        dma_xp(2); dma_xa(3); dma_xp(3); dma_xv(3)
        dma_xp(4); dma_xp(5); dma_xp(6)

        # ---------- ACT: load exp/ln table set once, before any activation --
        ldset = mybir.InstLoadActFuncSet(
            name=f"I-{nc.next_id()}", act_func_set_id=_act_set_id, ins=[], outs=[])
        ldset.engine = mybir.EngineType.Activation
        nc.scalar.add_instruction(ldset)

        # ---------- gpsimd consts ----------
        ones64 = ap.tile([64, 1], f32, tag="ones64")
        nc.gpsimd.memset(ones64[:], 1.0)
        ones16 = ap.tile([16, 1], f32, tag="ones16")
        nc.gpsimd.memset(ones16[:], 1.0)
        ones1x16 = ap.tile([1, 16], f32, tag="ones1x16")
        nc.gpsimd.memset(ones1x16[:], 1.0)
        rhs_sim = ap.tile([65, 4], f32, tag="rhs_sim")
        nc.gpsimd.memset(rhs_sim[64:65, :], 1.0)

        # ---------- sbuf activation tiles ----------
        pooled_v = ap.tile([128, 16], f32, tag="pooled_v")
        pooled_a = ap.tile([128, 16], f32, tag="pooled_a")
        pooled_dv = ap.tile([128, 8, 16], f32, tag="pooled_dv")
        scratch = ap.tile([128, 784], bf16, tag="scratch")
        xdb = ap.tile([128, 8, 2, 16], bf16, tag="xdb")   # b = 2j+s
        xmb = ap.tile([128, 16, 4], bf16, tag="xmb")
        y1o = ap.tile([128, 8, 16], bf16, tag="y1o")
        y2o = ap.tile([128, 4, 16], bf16, tag="y2o")
        origin = ap.tile([64, 16], f32, tag="origin")
        otmp = ap.tile([64, 16], f32, tag="otmp")
        q1 = ap.tile([64, 16], f32, tag="q1")
        qf = ap.tile([64, 16], f32, tag="qf")
        od = ap.tile([64, 16], f32, tag="od")
        osq = ap.tile([64, 16], f32, tag="osq")
        osvdd = ap.tile([1, 16], f32, tag="osvdd")
        y1s = ap.tile([128, 8, 16], bf16, tag="y1s")
        y2s = ap.tile([128, 4, 16], bf16, tag="y2s")
        stmp = ap.tile([64, 16], f32, tag="stmp")
        shallow = ap.tile([64, 16], f32, tag="shallow")
        cat1 = ap.tile([128, 16], f32, tag="cat1")
        cat2 = ap.tile([128, 16], f32, tag="cat2")
        t1 = ap.tile([64, 16], f32, tag="t1")
        sim_lhs = ap.tile([65, 16], f32, tag="sim_lhs")
        t2 = ap.tile([64, 16], f32, tag="t2")
        sim_sb = ap.tile([16, 4], f32, tag="sim_sb")
        m16 = ap.tile([16, 1], f32, tag="m16")
        negm = ap.tile([16, 1], f32, tag="negm")
        onehotT = ap.tile([16, 4], f32, tag="onehotT")
        oh_sb = ap.tile([4, 16], f32, tag="oh_sb")
        c1 = ap.tile([64, 16], f32, tag="c1")
        cf = ap.tile([64, 16], f32, tag="cf")
        cd = ap.tile([64, 16], f32, tag="cd")
        csq = ap.tile([64, 16], f32, tag="csq")
        csvdd = ap.tile([1, 16], f32, tag="csvdd")
        al = ap.tile([1, 16], f32, tag="al")
        pT2 = ap.tile([64, 4], f32, tag="pT2")
        pnorm = ap.tile([1, 4], f32, tag="pnorm")
        e_t = ap.tile([16, 4], f32, tag="e_t")
        s16 = ap.tile([16, 1], f32, tag="s16")
        ce_col = ap.tile([16, 1], f32, tag="ce_col")
        outv = ap.tile([1, 4], f32, tag="outv")

        # psum tiles
        pool_m = pmp.tile([128, 32], f32, tag="pool_m")       # col = ct*16+b
        pool_d = pdp.tile([128, 4, 16, 2], f32, tag="pool_d")  # [d%128, j, dc, s]

        ty1 = ap.tile([128, 8, 16], f32, tag="ty1")
        ty2 = ap.tile([128, 4, 16], f32, tag="ty2")
        tsm = ap.tile([64, 16], f32, tag="tsm")

        def lrelu(dst, src, tmp, scale=None):
            """dst = max(0.01*scale*src, scale*src) on DVE (2 ops, PSUM-safe)."""
            if scale is None:
                nc.vector.tensor_copy(tmp, src)
            else:
                nc.vector.tensor_scalar(tmp, src, scale, None, op0=ALU.mult)
            nc.vector.scalar_tensor_tensor(dst, tmp, 0.01, tmp,
                                           op0=ALU.mult, op1=ALU.max)

        # ---------- DVE consts (dep: blob64) ----------
        nc.vector.tensor_tensor(pT2[:], protoT, protoT, op=ALU.mult)
        nc.vector.tensor_scalar(rhs_sim[0:64, :], protoT, -2.0, None, op0=ALU.mult)

        # ---------- PE warm-up spin (dep: cat_blob ~2.3us) ----------
        warm_ps = pt.tile([128, 16], f32, tag="tail")
        for _ in range(48):
            nc.tensor.matmul(warm_ps[:, 0:1], catid, ones64[:],
                             start=True, stop=True)

        # ---------- PE: x_deep b8-15 pool (no DR; FWL-friendly 128-col) -----
        for t in range(64):  # t = j*16 + dc
            nc.tensor.matmul(pool_d[:, t // 16, t % 16, :],
                             xdP_t[:, 128 * t:128 * t + 128],
                             ones2_t[:], start=True, stop=True)
        pn_ps = pt.tile([128, 16], f32, tag="tail")
        nc.tensor.matmul(pn_ps[0:1, 0:4], ones64[:], pT2[:], start=True, stop=True)

        # ---------- DVE: x_deep b0-7 pool ----------
        for hf in range(2):
            nc.vector.reduce_sum(pooled_dv[:, 4 * hf:4 * hf + 4, :],
                                 xdV_t[:, 4 * hf:4 * hf + 4, :, :], axis=AX.X)
        # xdb evacs (bf16, fold 1/256/49)
        nc.vector.tensor_scalar(xdb[:, 0:4, :, :], pooled_dv[:], INV / 49.0,
                                None, op0=ALU.mult)
        for s in range(2):
            nc.vector.tensor_scalar(xdb[:, 4:8, s, :], pool_d[:, :, :, s],
                                    INV / 49.0, None, op0=ALU.mult)
        nc.vector.tensor_copy(pnorm[:], pn_ps[0:1, 0:4])

        # ---------- PE: origin layer 1 (k-outer, one psum bank) ----------
        y1o_ps = pbig.tile([128, 8, 16], f32, tag="big")
        for k in range(16):
            for m in range(8):
                nc.tensor.matmul(y1o_ps[:, m, :],
                                 ow1_t[:, k, 128 * m:128 * m + 128],
                                 xdb[:, :, :, k],
                                 start=(k == 0), stop=(k == 15))

        # ---------- ACT: x_mid ch 128-255 pool (16 lines, chunk-gated) ------
        for b in range(16):
            nc.scalar.activation(scratch[:], xmA_t[:, b, :], AF.Copy,
                                 accum_out=pooled_a[:, b:b + 1])
        nc.scalar.mul(xmb[:, :, 1], pooled_a[:], INV / 784.0)

        # ---------- DVE: x_mid ch 0-127 pools + origin-chain evacs ----------
        nc.vector.reduce_sum(pooled_v[:, 0:4], xmV_t[:, 0:4, :], axis=AX.X)
        lrelu(y1o[:], y1o_ps[:], ty1[:])

        def pool_mm(h, lim=32):
            """PE pool batch for xmP hw-tile h (32 MMs, chains over h)."""
            for t in range(lim):
                if h < 6:
                    nc.tensor.matmul(pool_m[:, t:t + 1],
                                     xmP_t[:, h, 128 * t:128 * t + 128],
                                     onescol_t[:], start=(h == 0), stop=False)
                else:
                    nc.tensor.matmul(pool_m[:, t:t + 1],
                                     xmP_t[0:16, 6, 128 * t:128 * t + 128],
                                     onescol_t[0:16, :], start=False, stop=True)

        # ---------- PE: y2o, interleaved with pool_m chunks ----------
        y2o_ps = pbig.tile([128, 4, 16], f32, tag="big")
        for k in range(8):
            for m in range(4):
                nc.tensor.matmul(y2o_ps[:, m, :],
                                 ow2_t[:, k, 128 * m:128 * m + 128],
                                 y1o[:, k, :], start=(k == 0), stop=(k == 7))
        pool_mm(0)

        nc.vector.reduce_sum(pooled_v[:, 4:8], xmV_t[:, 4:8, :], axis=AX.X)
        lrelu(y2o[:], y2o_ps[:], ty2[:])

        # origin = lrelu(psum * INV*INV)  (ow2,ow3 each carry x256)
        origin_ps = pt.tile([128, 16], f32, tag="tail")
        for k in range(4):
            nc.tensor.matmul(origin_ps[0:64, :], o3s3_t[:, k, 0:64],
                             y2o[:, k, :], start=(k == 0), stop=(k == 3))
        pool_mm(1)
        lrelu(origin[:], origin_ps[0:64, :], otmp[:], scale=INV2)

        # q chain (origin -> qf), interleaved on PE with pool chunks
        q1_ps = pt.tile([128, 16], f32, tag="tail")
        nc.tensor.matmul(q1_ps[0:64, :], qw1, origin[:], start=True, stop=True)
        pool_mm(2)
        lrelu(q1[:], q1_ps[0:64, :], tsm[:])
        q2_ps = pt.tile([128, 16], f32, tag="tail")
        nc.tensor.matmul(q2_ps[0:64, :], qw2, q1[:], start=True, stop=True)
        pool_mm(3)
        lrelu(qf[:], q2_ps[0:64, :], tsm[:])
        nc.vector.reduce_sum(pooled_v[:, 8:12], xmV_t[:, 8:12, :], axis=AX.X)
        nc.vector.tensor_scalar(od[:], qf[:], center_col, None, op0=ALU.subtract)
        nc.vector.tensor_tensor(osq[:], od[:], od[:], op=ALU.mult)
        osvdd_ps = pt.tile([128, 16], f32, tag="tail")
        nc.tensor.matmul(osvdd_ps[0:1, :], ones64[:], osq[:], start=True, stop=True)
        pool_mm(4)
        nc.vector.reduce_sum(pooled_v[:, 12:16], xmV_t[:, 12:16, :], axis=AX.X)
        nc.vector.tensor_copy(osvdd[:], osvdd_ps[0:1, :])
        nc.vector.tensor_scalar(xmb[:, :, 0], pooled_v[:], INV / 784.0, None,
                                op0=ALU.mult)
        pool_mm(5)
        pool_mm(6)

        # ---------- pool_m evac + M chain ----------
        for ct in range(2):
            nc.vector.tensor_scalar(xmb[:, :, 2 + ct],
                                    pool_m[:, 16 * ct:16 * ct + 16],
                                    INV / 784.0, None, op0=ALU.mult)
        y1s_ps = pbig.tile([128, 8, 16], f32, tag="big")
        for k in range(4):
            for m in range(8):
                nc.tensor.matmul(y1s_ps[:, m, :],
                                 MT_t[:, k, 128 * m:128 * m + 128],
                                 xmb[:, :, k], start=(k == 0), stop=(k == 3))
        lrelu(y1s[:], y1s_ps[:], ty1[:])
        y2s_ps = pbig.tile([128, 4, 16], f32, tag="big")
        for k in range(8):
            for m in range(4):
                nc.tensor.matmul(y2s_ps[:, m, :],
                                 sw2_t[:, k, 128 * m:128 * m + 128],
                                 y1s[:, k, :], start=(k == 0), stop=(k == 7))
        lrelu(y2s[:], y2s_ps[:], ty2[:])
        sh_ps = pt.tile([128, 16], f32, tag="tail")
        for k in range(4):
            nc.tensor.matmul(sh_ps[0:64, :], o3s3_t[:, k, 64:128],
                             y2s[:, k, :], start=(k == 0), stop=(k == 3))
        lrelu(shallow[:], sh_ps[0:64, :], stmp[:], scale=INV2)

        # ---------- texture path ----------
        cat1_ps = pt.tile([128, 16], f32, tag="tail")
        nc.tensor.matmul(cat1_ps[:], catid, shallow[:], start=True, stop=False)
        nc.tensor.matmul(cat1_ps[:], neg_cpad, ones1x16[:], start=False, stop=True)
        nc.vector.tensor_copy(cat1[:], cat1_ps[:])
        t1_ps = pt.tile([128, 16], f32, tag="tail")
        nc.tensor.matmul(t1_ps[0:64, :], tw1, cat1[:], start=True, stop=True)
        lrelu(t1[:], t1_ps[0:64, :], tsm[:])
        t2_ps = pt.tile([128, 16], f32, tag="tail")
        nc.tensor.matmul(t2_ps[0:64, :], tw2, t1[:], start=True, stop=True)
        lrelu(sim_lhs[0:64, :], t2_ps[0:64, :], tsm[:])

        # ---------- sim + argmax + CE ----------
        nc.vector.tensor_tensor(t2[:], sim_lhs[0:64, :], sim_lhs[0:64, :],
                                op=ALU.mult)
        tsq_ps = pt.tile([128, 16], f32, tag="tail")
        nc.tensor.matmul(tsq_ps[0:1, :], ones64[:], t2[:], start=True, stop=True)
        nc.vector.tensor_copy(sim_lhs[64:65, :], tsq_ps[0:1, :])
        sim_ps = pt.tile([128, 16], f32, tag="tail")
        nc.tensor.matmul(sim_ps[0:16, 0:4], sim_lhs[:], rhs_sim[:],
                         start=True, stop=False)
        nc.tensor.matmul(sim_ps[0:16, 0:4], ones1x16[:], pnorm[:],
                         start=False, stop=True)
        nc.vector.tensor_copy(sim_sb[:], sim_ps[0:16, 0:4])
        nc.vector.reduce_max(m16[:], sim_sb[:], axis=AX.X)
        nc.vector.reduce_max(negm[:], sim_sb[:], axis=AX.X, negate=True)
        nc.vector.tensor_scalar(onehotT[:], sim_sb[:], m16[:, 0:1], None,
                                op0=ALU.is_ge)
        # CE on ACT (table preloaded at t0)
        nc.scalar.activation(e_t[:], sim_sb[:], AF.Exp, bias=negm[:, 0:1],
                             accum_out=s16[:])
        nc.scalar.activation(ce_col[:], s16[:], AF.Ln)
        oh_ps = pt.tile([128, 16], f32, tag="tail")
        nc.tensor.transpose(oh_ps[0:4, 0:16], onehotT[:], id16_t[:])
        nc.vector.tensor_copy(oh_sb[:], oh_ps[0:4, 0:16])

        # ---------- class feat chain ----------
        cat2_ps = pt.tile([128, 16], f32, tag="tail")
        nc.tensor.matmul(cat2_ps[:], catid, origin[:], start=True, stop=False)
        nc.tensor.matmul(cat2_ps[:], neg_ppad, oh_sb[:], start=False, stop=True)
        ce_ps = pt.tile([128, 16], f32, tag="tail")
        nc.tensor.matmul(ce_ps[0:1, 0:1], ce_col[:], ones16[:],
                         start=True, stop=True)
        nc.vector.tensor_copy(cat2[:], cat2_ps[:])
        cw1_ps = pt.tile([128, 16], f32, tag="tail")
        nc.tensor.matmul(cw1_ps[0:64, :], cw1, cat2[:], start=True, stop=True)
        lrelu(c1[:], cw1_ps[0:64, :], tsm[:])
        cw2_ps = pt.tile([128, 16], f32, tag="tail")
        nc.tensor.matmul(cw2_ps[0:64, :], cw2, c1[:], start=True, stop=True)
        lrelu(cf[:], cw2_ps[0:64, :], tsm[:])
        nc.vector.tensor_scalar(cd[:], cf[:], center_col, None, op0=ALU.subtract)
        nc.vector.tensor_tensor(csq[:], cd[:], cd[:], op=ALU.mult)
        csvdd_ps = pt.tile([128, 16], f32, tag="tail")
        nc.tensor.matmul(csvdd_ps[0:1, :], ones64[:], csq[:], start=True, stop=True)
        nc.vector.tensor_copy(csvdd[:], csvdd_ps[0:1, :])

        # ---------- align + output ----------
        nc.vector.tensor_tensor(al[:], osvdd[:], csvdd[:], op=ALU.subtract)
        nc.vector.scalar_tensor_tensor(al[:], al[:], -1.0, al[:],
                                       op0=ALU.mult, op1=ALU.max)
        nc.vector.tensor_copy(outv[0:1, 0:1], ce_ps[0:1, 0:1])
        nc.vector.reduce_sum(outv[0:1, 1:2], osvdd[:], axis=AX.X)
        nc.vector.reduce_sum(outv[0:1, 2:3], csvdd[:], axis=AX.X)
        nc.vector.reduce_sum(outv[0:1, 3:4], al[:], axis=AX.X)
        nc.sync.dma_start(out=out_d[:], in_=outv[:])

    nc.compile()
    return nc


def _host_prep(inputs):
    f = np.float32
    xm8 = np.asarray(inputs["x_mid"], f).reshape(B, 512, 784).astype(F8)
    xd8 = np.asarray(inputs["x_deep"], f).reshape(B, 2048, 49).astype(F8)

    def T(w):
        return np.ascontiguousarray(np.asarray(w, f).T)

    def T8(w):
        return (T(w) * WSCALE).astype(F8)

    def ptile(w, kk):  # [K, O] -> [128, kk, O] with row k*128+p -> [p, k, :]
        K, O = w.shape
        return np.ascontiguousarray(w.reshape(kk, 128, O).transpose(1, 0, 2))

    M = np.asarray(inputs["w_shallow"], f).T @ np.asarray(inputs["sw1"], f).T

    center = np.asarray(inputs["center"], f)
    proto = np.asarray(inputs["proto"], f)
    catid = np.zeros((64, 128), f)
    catid[np.arange(64), np.arange(64)] = 1
    catid[np.arange(64), 64 + np.arange(64)] = 1
    ppad = np.zeros((4, 128), f)
    ppad[:, 64:] = -proto
    cpad = np.zeros((1, 128), f)
    cpad[0, 64:] = -center
    ones2 = np.zeros((98, 2), dtype=BF)
    ones2[0:49, 0] = 1
    ones2[49:98, 1] = 1
    o3s3 = np.concatenate([ptile(T8(inputs["ow3"]), 4),
                           ptile(T8(inputs["sw3"]), 4)], axis=2)
    blob128 = np.concatenate([T(inputs["tw1"]), T(inputs["cw1"])], axis=1)
    blob64 = np.concatenate(
        [T(inputs["tw2"]), T(inputs["cw2"]), T(inputs["qw1"]),
         T(inputs["qw2"]), T(proto), center.reshape(64, 1)], axis=1)

    shared = {
        "ow1T": ptile(T8(inputs["ow1"]), 16),
        "MT": ptile((M * WSCALE).astype(F8), 4),
        "ow2T": ptile(T8(inputs["ow2"]), 8),
        "sw2T": ptile(T8(inputs["sw2"]), 8),
        "o3s3": np.ascontiguousarray(o3s3),
        "blob128": np.ascontiguousarray(blob128),
        "blob64": np.ascontiguousarray(blob64),
        "cat_blob": catid,
        "ppad": ppad,
        "cpad": cpad,
        "id16": np.eye(16, dtype=f),
        "ones2": ones2,
        "onescol": np.ones((128, 1), dtype=BF),
    }
    in_maps = []
    for c in range(N_CORES):
        m = dict(shared)
        xc = xm8[c * BC:(c + 1) * BC]          # [16, 512, 784]
        xdc = xd8[c * BC:(c + 1) * BC]         # [16, 2048, 49]
        m["xmV"] = np.ascontiguousarray(xc[:, 0:128].transpose(1, 0, 2))
        m["xmA"] = np.ascontiguousarray(xc[:, 128:256].transpose(1, 0, 2))
        # xmP: [hw, (ct, b, c_lo)]
        m["xmP"] = np.ascontiguousarray(
            xc[:, 256:512].reshape(16, 2, 128, 784)
            .transpose(3, 1, 0, 2)).reshape(784, 4096)
        # xdV: b0-7, [d%128, b, d//128, hw]
        m["xdV"] = np.ascontiguousarray(
            xdc[0:8].reshape(8, 16, 128, 49).transpose(2, 0, 1, 3))
        # xdP: b8-15 packed 2 samples per partition set (even b upper, odd lower)
        hi = xdc[8:16]                          # [8, 2048, 49]
        ev = hi[0::2].transpose(2, 0, 1)        # [49, 4, 2048]
        od = hi[1::2].transpose(2, 0, 1)
        m["xdP"] = np.ascontiguousarray(
            np.concatenate([ev, od], axis=0)).reshape(98, 8192)
        in_maps.append(m)
    return in_maps


def _get_program():
    if "nc" not in _CACHE:
        _CACHE["nc"] = _build_program()
    return _CACHE["nc"]


def _combine(parts):
    tot = np.sum([np.asarray(p, np.float64).ravel() for p in parts], axis=0)
    return (tot / B).astype(np.float32).reshape(4, 1)


def _run(inputs, trace=False):
    from concourse.bass_utils import run_bass_kernel_spmd
    nc = _get_program()
    in_maps = _host_prep(inputs)
    kw = {}
    if trace:
        kw = dict(trace=True, trace_cores=list(range(N_CORES)))
    res = run_bass_kernel_spmd(nc, in_maps, list(range(N_CORES)), **kw)
    out = _combine([res.results[i]["out"] for i in range(N_CORES)])
    return out, res


def kernel(**inputs):
    out, _ = _run(inputs, trace=False)
    return out


def kernel_traced(**inputs):
    """Returns (output, exec_time_ns) using the NTFF profile (max over cores)."""
    out, res = _run(inputs, trace=True)
    return out, res.exec_time_ns


# revision 6
# speedup vs baseline: 1.1941x; 1.1219x over previous
"""DGAD net (vq_codebook) kernel v4 for 8x Trainium2 NeuronCores.

Contract: kernel(**inputs) takes FULL unsharded inputs, returns FULL [4,1]
fp32 output. Batch (128) sharded 16/core; weights replicated; final
all-reduce (sum/128) on host during unshard.

v4 vs v3 (85.4us):
  - No DoubleRow on PE pools (DR is a net loss at free-dim<128: ~120ns/MM
    vs ~40ns; LDWEIGHTS scales with cols, not rows).
  - PE queue reordered: origin chain (ow1/ow2/ow3/qw, x_deep-only deps)
    runs DURING x_mid streaming, interleaved with pool_m chunk batches,
    instead of after all pools.
  - Two HWDGE rings (Sync + ACT) issue DMAs; chunks interleaved so each
    pool engine streams as data arrives. x-inputs chunked 4-8 ways.
  - All lrelu evacs on DVE via scalar_tensor_tensor (0.01*x max x); ACT
    keeps the Exp/Ln table set loaded from t=0 (no mid-tail table switch).
  - Per-layer matmuls accumulate into ONE psum bank region ([128,8,16]),
    single DVE evac, instead of per-m-chunk psum tiles + evacs.
  - Small weights packed into 4 blob DMAs (blob128/blob64/cat_blob/id16).
"""

import numpy as np
import ml_dtypes

N_CORES = 8
B = 128
BC = B // N_CORES  # 16 samples per core

BF = ml_dtypes.bfloat16
F8 = ml_dtypes.float8_e4m3
WSCALE = 256.0  # fp8 weights stored *256; 1/256 folded into consumer scales

_CACHE = {}


def _build_program():
    import concourse.bass as bass  # noqa: F401
    import concourse.mybir as mybir
    import concourse.tile as tile
    from concourse import bacc
    from contextlib import ExitStack

    dt = mybir.dt
    AF = mybir.ActivationFunctionType
    ALU = mybir.AluOpType
    AX = mybir.AxisListType
    f32, bf16, f8 = dt.float32, dt.bfloat16, dt.float8e4
    INV = 1.0 / WSCALE
    INV2 = INV * INV

    from concourse.hw_specs import get_activation_tables
    _act_set_id = list(get_activation_tables("gen3")).index("natural_log_exp_and_others")

    nc = bacc.Bacc("TRN2", target_bir_lowering=False, debug=False,
                   enable_asserts=True, num_devices=N_CORES)

    def din(name, shape, d):
        return nc.dram_tensor(name, shape, d, kind="ExternalInput").ap()

    xmV_d = din("xmV", [128, 16, 784], f8)    # ch 0-127, [c,b,hw] (DVE)
    xmA_d = din("xmA", [128, 16, 784], f8)    # ch 128-255, [c,b,hw] (ACT)
    xmP_d = din("xmP", [784, 4096], f8)       # ch 256-511, [hw,(ct,b,c_lo)] (PE)
    xdV_d = din("xdV", [128, 8, 16, 49], f8)  # b0-7, [d%128, b, d//128, hw] (DVE)
    xdP_d = din("xdP", [98, 8192], f8)        # b8-15 2-packed, [hw(+49*par), j*2048+d] (PE)
    ow1T_d = din("ow1T", [128, 16, 1024], f8)   # (k p) o -> p k o, *256
    MT_d = din("MT", [128, 4, 1024], f8)        # (wsh.T @ sw1.T)*256, pre-permuted
    ow2T_d = din("ow2T", [128, 8, 512], f8)
    sw2T_d = din("sw2T", [128, 8, 512], f8)
    o3s3_d = din("o3s3", [128, 4, 128], f8)     # cols 0:64 ow3T, 64:128 sw3T
    blob128_d = din("blob128", [128, 128], f32)  # cols 0:64 tw1T, 64:128 cw1T
    blob64_d = din("blob64", [64, 261], f32)     # tw2|cw2|qw1|qw2|protoT|center
    cat_blob_d = din("cat_blob", [64, 128], f32)  # catid
    ppad_d = din("ppad", [4, 128], f32)           # cols 64: = -proto
    cpad_d = din("cpad", [1, 128], f32)           # cols 64: = -center
    id16_d = din("id16", [16, 16], f32)
    ones2_d = din("ones2", [98, 2], bf16)       # [:49]=[1,0], [49:]=[0,1]
    onescol_d = din("onescol", [128, 1], bf16)
    out_d = nc.dram_tensor("out", [1, 4], f32, kind="ExternalOutput").ap()

    with tile.TileContext(nc) as tc, ExitStack() as ctx:
        wp = ctx.enter_context(tc.tile_pool(name="wp", bufs=1))
        xp = ctx.enter_context(tc.tile_pool(name="xp", bufs=1))
        ap = ctx.enter_context(tc.tile_pool(name="ap", bufs=1))
        pmp = ctx.enter_context(tc.tile_pool(name="pmp", bufs=1, space="PSUM"))
        pdp = ctx.enter_context(tc.tile_pool(name="pdp", bufs=1, space="PSUM"))
        pbig = ctx.enter_context(tc.tile_pool(name="pbig", bufs=2, space="PSUM"))
        pt = ctx.enter_context(tc.tile_pool(name="pt", bufs=3, space="PSUM"))

        # ---------- weight tiles ----------
        ow1_t = wp.tile([128, 16, 1024], f8, tag="ow1")
        MT_t = wp.tile([128, 4, 1024], f8, tag="MT")
        ow2_t = wp.tile([128, 8, 512], f8, tag="ow2")
        sw2_t = wp.tile([128, 8, 512], f8, tag="sw2")
        o3s3_t = wp.tile([128, 4, 128], f8, tag="o3s3")
        blob128_t = wp.tile([128, 128], f32, tag="blob128")
        blob64_t = wp.tile([64, 261], f32, tag="blob64")
        cat_blob_t = wp.tile([64, 128], f32, tag="cat_blob")
        ppad_t = wp.tile([4, 128], f32, tag="ppad")
        cpad_t = wp.tile([1, 128], f32, tag="cpad")
        id16_t = wp.tile([16, 16], f32, tag="id16")
        ones2_t = wp.tile([98, 2], bf16, tag="ones2")
        onescol_t = wp.tile([128, 1], bf16, tag="onescol")

        xmV_t = xp.tile([128, 16, 784], f8, tag="xmV")
        xmA_t = xp.tile([128, 16, 784], f8, tag="xmA")
        xmP_t = xp.tile([128, 7, 4096], f8, tag="xmP")
        xdV_t = xp.tile([128, 8, 16, 49], f8, tag="xdV")
        xdP_t = xp.tile([98, 8192], f8, tag="xdP")

        # weight slices
        tw1 = blob128_t[:, 0:64]
        cw1 = blob128_t[:, 64:128]
        tw2 = blob64_t[:, 0:64]
        cw2 = blob64_t[:, 64:128]
        qw1 = blob64_t[:, 128:192]
        qw2 = blob64_t[:, 192:256]
        protoT = blob64_t[:, 256:260]
        center_col = blob64_t[:, 260:261]
        catid = cat_blob_t[:]
        neg_ppad = ppad_t[:]
        neg_cpad = cpad_t[:]

        # ---------- ACT: load exp/ln table set once, before anything ------
        ldset = mybir.InstLoadActFuncSet(
            name=f"I-{nc.next_id()}", act_func_set_id=_act_set_id, ins=[], outs=[])
        ldset.engine = mybir.EngineType.Activation
        nc.scalar.add_instruction(ldset)

        # ---------- DMA issue: ACT ring (slow ~20GB/s -> tiny blobs only) ---
        for t_, d_ in ((ones2_t, ones2_d), (onescol_t, onescol_d),
                       (cat_blob_t, cat_blob_d), (blob128_t, blob128_d),
                       (blob64_t, blob64_d), (ppad_t, ppad_d),
                       (cpad_t, cpad_d), (id16_t, id16_d)):
            nc.scalar.dma_start(out=t_[:], in_=d_)

        # ---------- DMA issue: Sync ring (19), interleaved arrival order ----
        def dma_ow1(k):
            nc.sync.dma_start(out=ow1_t[:, 4 * k:4 * k + 4, :],
                              in_=ow1T_d[:, 4 * k:4 * k + 4, :])

        def dma_xa(q):
            nc.sync.dma_start(out=xmA_t[:, 4 * q:4 * q + 4, :],
                              in_=xmA_d[:, 4 * q:4 * q + 4, :])

        def dma_xv(q):
            nc.sync.dma_start(out=xmV_t[:, 4 * q:4 * q + 4, :],
                              in_=xmV_d[:, 4 * q:4 * q + 4, :])

        def dma_xp(h):
            if h < 6:
                nc.sync.dma_start(out=xmP_t[:, h, :],
                                  in_=xmP_d[128 * h:128 * h + 128, :])
            else:
                nc.sync.dma_start(out=xmP_t[0:16, 6, :], in_=xmP_d[768:784, :])

        nc.sync.dma_start(out=xdV_t[:, 0:4, :, :], in_=xdV_d[:, 0:4, :, :])
        nc.sync.dma_start(out=xdV_t[:, 4:8, :, :], in_=xdV_d[:, 4:8, :, :])
        nc.sync.dma_start(out=xdP_t[:], in_=xdP_d)
        dma_ow1(0); dma_ow1(1); dma_ow1(2); dma_ow1(3)
        nc.sync.dma_start(out=ow2_t[:], in_=ow2T_d)
        nc.sync.dma_start(out=o3s3_t[:], in_=o3s3_d)
        dma_xa(0); dma_xv(0); dma_xp(0)
        dma_xa(1); dma_xv(1); dma_xp(1)
        dma_xa(2); dma_xv(2); dma_xp(2)
        dma_xa(3); dma_xv(3); dma_xp(3)
        dma_xp(4)
        nc.sync.dma_start(out=MT_t[:], in_=MT_d)
        dma_xp(5); dma_xp(6)
        nc.sync.dma_start(out=sw2_t[:], in_=sw2T_d)

        # ---------- gpsimd consts ----------
        ones64 = ap.tile([64, 1], f32, tag="ones64")
        nc.gpsimd.memset(ones64[:], 1.0)
        ones16 = ap.tile([16, 1], f32, tag="ones16")
        nc.gpsimd.memset(ones16[:], 1.0)
        ones1x16 = ap.tile([1, 16], f32, tag="ones1x16")
        nc.gpsimd.memset(ones1x16[:], 1.0)
        rhs_sim = ap.tile([65, 4], f32, tag="rhs_sim")
        nc.gpsimd.memset(rhs_sim[64:65, :], 1.0)

        # ---------- sbuf activation tiles ----------
        pooled_v = ap.tile([128, 16], f32, tag="pooled_v")
        pooled_a = ap.tile([128, 16], f32, tag="pooled_a")
        pooled_dv = ap.tile([128, 8, 16], f32, tag="pooled_dv")
        scratch = ap.tile([128, 784], bf16, tag="scratch")
        xdb = ap.tile([128, 8, 2, 16], bf16, tag="xdb")   # b = 2j+s
        xmb = ap.tile([128, 16, 4], bf16, tag="xmb")
        y1o = ap.tile([128, 8, 16], bf16, tag="y1o")
        y2o = ap.tile([128, 4, 16], bf16, tag="y2o")
        origin = ap.tile([64, 16], f32, tag="origin")
        otmp = ap.tile([64, 16], f32, tag="otmp")
        q1 = ap.tile([64, 16], f32, tag="q1")
        qf = ap.tile([64, 16], f32, tag="qf")
        od = ap.tile([64, 16], f32, tag="od")
        osq = ap.tile([64, 16], f32, tag="osq")
        osvdd = ap.tile([1, 16], f32, tag="osvdd")
        y1s = ap.tile([128, 8, 16], bf16, tag="y1s")
        y2s = ap.tile([128, 4, 16], bf16, tag="y2s")
        stmp = ap.tile([64, 16], f32, tag="stmp")
        shallow = ap.tile([64, 16], f32, tag="shallow")
        cat1 = ap.tile([128, 16], f32, tag="cat1")
        cat2 = ap.tile([128, 16], f32, tag="cat2")
        t1 = ap.tile([64, 16], f32, tag="t1")
        sim_lhs = ap.tile([65, 16], f32, tag="sim_lhs")
        t2 = ap.tile([64, 16], f32, tag="t2")
        sim_sb = ap.tile([16, 4], f32, tag="sim_sb")
        m16 = ap.tile([16, 1], f32, tag="m16")
        negm = ap.tile([16, 1], f32, tag="negm")
        onehotT = ap.tile([16, 4], f32, tag="onehotT")
        oh_sb = ap.tile([4, 16], f32, tag="oh_sb")
        c1 = ap.tile([64, 16], f32, tag="c1")
        cf = ap.tile([64, 16], f32, tag="cf")
        cd = ap.tile([64, 16], f32, tag="cd")
        csq = ap.tile([64, 16], f32, tag="csq")
        csvdd = ap.tile([1, 16], f32, tag="csvdd")
        al = ap.tile([1, 16], f32, tag="al")
        pT2 = ap.tile([64, 4], f32, tag="pT2")
        pnorm = ap.tile([1, 4], f32, tag="pnorm")
        e_t = ap.tile([16, 4], f32, tag="e_t")
        s16 = ap.tile([16, 1], f32, tag="s16")
        ce_col = ap.tile([16, 1], f32, tag="ce_col")
        outv = ap.tile([1, 4], f32, tag="outv")

        # psum tiles
        pool_m = pmp.tile([128, 32], f32, tag="pool_m")       # col = ct*16+b
        pool_d = pdp.tile([128, 4, 16, 2], f32, tag="pool_d")  # [d%128, j, dc, s]

        ty1 = ap.tile([128, 8, 16], f32, tag="ty1")
        ty2 = ap.tile([128, 4, 16], f32, tag="ty2")
        tsm = ap.tile([64, 16], f32, tag="tsm")

        def lrelu(dst, src, tmp, scale=None):
            """dst = max(0.01*scale*src, scale*src) on DVE (2 ops, PSUM-safe)."""
            if scale is None:
                nc.vector.tensor_copy(tmp, src)
            else:
                nc.vector.tensor_scalar(tmp, src, scale, None, op0=ALU.mult)
            nc.vector.scalar_tensor_tensor(dst, tmp, 0.01, tmp,
                                           op0=ALU.mult, op1=ALU.max)

        # ---------- DVE consts (dep: blob64) ----------
        nc.vector.tensor_tensor(pT2[:], protoT, protoT, op=ALU.mult)
        nc.vector.tensor_scalar(rhs_sim[0:64, :], protoT, -2.0, None, op0=ALU.mult)

        # ---------- PE warm-up spin (dep: cat_blob ~2.3us) ----------
        warm_ps = pt.tile([128, 16], f32, tag="tail")
        for _ in range(12):
            nc.tensor.matmul(warm_ps[0:2, 0:2], ones2_t[:], ones2_t[:],
                             start=True, stop=True)

        # ---------- PE: x_deep b8-15 pool (no DR; FWL-friendly 128-col) -----
        for t in range(64):  # t = j*16 + dc
            nc.tensor.matmul(pool_d[:, t // 16, t % 16, :],
                             xdP_t[:, 128 * t:128 * t + 128],
                             ones2_t[:], start=True, stop=True)
        pn_ps = pt.tile([128, 16], f32, tag="tail")
        nc.tensor.matmul(pn_ps[0:1, 0:4], ones64[:], pT2[:], start=True, stop=True)

        # ---------- DVE: x_deep b0-7 pool ----------
        for hf in range(2):
            nc.vector.reduce_sum(pooled_dv[:, 4 * hf:4 * hf + 4, :],
                                 xdV_t[:, 4 * hf:4 * hf + 4, :, :], axis=AX.X)
        # xdb evacs (bf16, fold 1/256/49)
        nc.vector.tensor_scalar(xdb[:, 0:4, :, :], pooled_dv[:], INV / 49.0,
                                None, op0=ALU.mult)
        for s in range(2):
            nc.vector.tensor_scalar(xdb[:, 4:8, s, :], pool_d[:, :, :, s],
                                    INV / 49.0, None, op0=ALU.mult)
        nc.vector.tensor_copy(pnorm[:], pn_ps[0:1, 0:4])

        # ---------- PE: origin layer 1 (k-outer, one psum bank) ----------
        y1o_ps = pbig.tile([128, 8, 16], f32, tag="big")
        for k in range(16):
            for m in range(8):
                nc.tensor.matmul(y1o_ps[:, m, :],
                                 ow1_t[:, k, 128 * m:128 * m + 128],
                                 xdb[:, :, :, k],
                                 start=(k == 0), stop=(k == 15))

        # ---------- ACT: x_mid ch 128-255 pool (16 lines, chunk-gated) ------
        for b in range(16):
            nc.scalar.activation(scratch[:], xmA_t[:, b, :], AF.Copy,
                                 accum_out=pooled_a[:, b:b + 1])
        nc.scalar.mul(xmb[:, :, 1], pooled_a[:], INV / 784.0)

        # ---------- DVE: x_mid ch 0-127 pools + origin-chain evacs ----------
        nc.vector.reduce_sum(pooled_v[:, 0:4], xmV_t[:, 0:4, :], axis=AX.X)
        lrelu(y1o[:], y1o_ps[:], ty1[:])

        def pool_mm(h, lim=32):
            """PE pool batch for xmP hw-tile h (32 MMs, chains over h)."""
            for t in range(lim):
                if h < 6:
                    nc.tensor.matmul(pool_m[:, t:t + 1],
                                     xmP_t[:, h, 128 * t:128 * t + 128],
                                     onescol_t[:], start=(h == 0), stop=False)
                else:
                    nc.tensor.matmul(pool_m[:, t:t + 1],
                                     xmP_t[0:16, 6, 128 * t:128 * t + 128],
                                     onescol_t[0:16, :], start=False, stop=True)

        # ---------- PE: y2o, interleaved with pool_m chunks ----------
        y2o_ps = pbig.tile([128, 4, 16], f32, tag="big")
        for k in range(8):
            for m in range(4):
                nc.tensor.matmul(y2o_ps[:, m, :],
                                 ow2_t[:, k, 128 * m:128 * m + 128],
                                 y1o[:, k, :], start=(k == 0), stop=(k == 7))
        pool_mm(0)

        nc.vector.reduce_sum(pooled_v[:, 4:8], xmV_t[:, 4:8, :], axis=AX.X)
        lrelu(y2o[:], y2o_ps[:], ty2[:])

        # origin = lrelu(psum * INV*INV)  (ow2,ow3 each carry x256)
        origin_ps = pt.tile([128, 16], f32, tag="tail")
        for k in range(4):
            nc.tensor.matmul(origin_ps[0:64, :], o3s3_t[:, k, 0:64],
                             y2o[:, k, :], start=(k == 0), stop=(k == 3))
        pool_mm(1)
        lrelu(origin[:], origin_ps[0:64, :], otmp[:], scale=INV2)

        # q chain (origin -> qf), interleaved on PE with pool chunks
        q1_ps = pt.tile([128, 16], f32, tag="tail")
        nc.tensor.matmul(q1_ps[0:64, :], qw1, origin[:], start=True, stop=True)
        pool_mm(2)
        lrelu(q1[:], q1_ps[0:64, :], tsm[:])
        q2_ps = pt.tile([128, 16], f32, tag="tail")
        nc.tensor.matmul(q2_ps[0:64, :], qw2, q1[:], start=True, stop=True)
        pool_mm(3)
        lrelu(qf[:], q2_ps[0:64, :], tsm[:])
        nc.vector.reduce_sum(pooled_v[:, 8:12], xmV_t[:, 8:12, :], axis=AX.X)
        nc.vector.tensor_scalar(od[:], qf[:], center_col, None, op0=ALU.subtract)
        nc.vector.tensor_tensor(osq[:], od[:], od[:], op=ALU.mult)
        osvdd_ps = pt.tile([128, 16], f32, tag="tail")
        nc.tensor.matmul(osvdd_ps[0:1, :], ones64[:], osq[:], start=True, stop=True)
        pool_mm(4)
        nc.vector.reduce_sum(pooled_v[:, 12:16], xmV_t[:, 12:16, :], axis=AX.X)
        nc.vector.tensor_copy(osvdd[:], osvdd_ps[0:1, :])
        nc.vector.tensor_scalar(xmb[:, :, 0], pooled_v[:], INV / 784.0, None,
                                op0=ALU.mult)
        pool_mm(5)
        pool_mm(6)

        # ---------- pool_m evac + M chain ----------
        for ct in range(2):
            nc.vector.tensor_scalar(xmb[:, :, 2 + ct],
                                    pool_m[:, 16 * ct:16 * ct + 16],
                                    INV / 784.0, None, op0=ALU.mult)
        y1s_ps = pbig.tile([128, 8, 16], f32, tag="big")
        for k in range(4):
            for m in range(8):
                nc.tensor.matmul(y1s_ps[:, m, :],
                                 MT_t[:, k, 128 * m:128 * m + 128],
                                 xmb[:, :, k], start=(k == 0), stop=(k == 3))
        lrelu(y1s[:], y1s_ps[:], ty1[:])
        y2s_ps = pbig.tile([128, 4, 16], f32, tag="big")
        for k in range(8):
            for m in range(4):
                nc.tensor.matmul(y2s_ps[:, m, :],
                                 sw2_t[:, k, 128 * m:128 * m + 128],
                                 y1s[:, k, :], start=(k == 0), stop=(k == 7))
        lrelu(y2s[:], y2s_ps[:], ty2[:])
        sh_ps = pt.tile([128, 16], f32, tag="tail")
        for k in range(4):
            nc.tensor.matmul(sh_ps[0:64, :], o3s3_t[:, k, 64:128],
                             y2s[:, k, :], start=(k == 0), stop=(k == 3))
        lrelu(shallow[:], sh_ps[0:64, :], stmp[:], scale=INV2)

        # ---------- texture path ----------
        cat1_ps = pt.tile([128, 16], f32, tag="tail")
        nc.tensor.matmul(cat1_ps[:], catid, shallow[:], start=True, stop=False)
        nc.tensor.matmul(cat1_ps[:], neg_cpad, ones1x16[:], start=False, stop=True)
        nc.vector.tensor_copy(cat1[:], cat1_ps[:])
        t1_ps = pt.tile([128, 16], f32, tag="tail")
        nc.tensor.matmul(t1_ps[0:64, :], tw1, cat1[:], start=True, stop=True)
        lrelu(t1[:], t1_ps[0:64, :], tsm[:])
        t2_ps = pt.tile([128, 16], f32, tag="tail")
        nc.tensor.matmul(t2_ps[0:64, :], tw2, t1[:], start=True, stop=True)
        lrelu(sim_lhs[0:64, :], t2_ps[0:64, :], tsm[:])

        # ---------- sim + argmax + CE ----------
        nc.vector.tensor_tensor(t2[:], sim_lhs[0:64, :], sim_lhs[0:64, :],
                                op=ALU.mult)
        tsq_ps = pt.tile([128, 16], f32, tag="tail")
        nc.tensor.matmul(tsq_ps[0:1, :], ones64[:], t2[:], start=True, stop=True)
        nc.vector.tensor_copy(sim_lhs[64:65, :], tsq_ps[0:1, :])
        sim_ps = pt.tile([128, 16], f32, tag="tail")
        nc.tensor.matmul(sim_ps[0:16, 0:4], sim_lhs[:], rhs_sim[:],
                         start=True, stop=False)
        nc.tensor.matmul(sim_ps[0:16, 0:4], ones1x16[:], pnorm[:],
                         start=False, stop=True)
        nc.vector.tensor_copy(sim_sb[:], sim_ps[0:16, 0:4])
        nc.vector.reduce_max(m16[:], sim_sb[:], axis=AX.X)
        nc.vector.reduce_max(negm[:], sim_sb[:], axis=AX.X, negate=True)
        nc.vector.tensor_scalar(onehotT[:], sim_sb[:], m16[:, 0:1], None,
                                op0=ALU.is_ge)
        # CE on ACT (table preloaded at t0)
        nc.scalar.activation(e_t[:], sim_sb[:], AF.Exp, bias=negm[:, 0:1],
                             accum_out=s16[:])
        nc.scalar.activation(ce_col[:], s16[:], AF.Ln)
        oh_ps = pt.tile([128, 16], f32, tag="tail")
        nc.tensor.transpose(oh_ps[0:4, 0:16], onehotT[:], id16_t[:])
        nc.vector.tensor_copy(oh_sb[:], oh_ps[0:4, 0:16])

        # ---------- class feat chain ----------
        cat2_ps = pt.tile([128, 16], f32, tag="tail")
        nc.tensor.matmul(cat2_ps[:], catid, origin[:], start=True, stop=False)
        nc.tensor.matmul(cat2_ps[:], neg_ppad, oh_sb[:], start=False, stop=True)
        ce_ps = pt.tile([128, 16], f32, tag="tail")
        nc.tensor.matmul(ce_ps[0:1, 0:1], ce_col[:], ones16[:],
                         start=True, stop=True)
        nc.vector.tensor_copy(cat2[:], cat2_ps[:])
        cw1_ps = pt.tile([128, 16], f32, tag="tail")
        nc.tensor.matmul(cw1_ps[0:64, :], cw1, cat2[:], start=True, stop=True)
        lrelu(c1[:], cw1_ps[0:64, :], tsm[:])
        cw2_ps = pt.tile([128, 16], f32, tag="tail")
        nc.tensor.matmul(cw2_ps[0:64, :], cw2, c1[:], start=True, stop=True)
        lrelu(cf[:], cw2_ps[0:64, :], tsm[:])
        nc.vector.tensor_scalar(cd[:], cf[:], center_col, None, op0=ALU.subtract)
        nc.vector.tensor_tensor(csq[:], cd[:], cd[:], op=ALU.mult)
        csvdd_ps = pt.tile([128, 16], f32, tag="tail")
        nc.tensor.matmul(csvdd_ps[0:1, :], ones64[:], csq[:], start=True, stop=True)
        nc.vector.tensor_copy(csvdd[:], csvdd_ps[0:1, :])

        # ---------- align + output ----------
        nc.vector.tensor_tensor(al[:], osvdd[:], csvdd[:], op=ALU.subtract)
        nc.vector.scalar_tensor_tensor(al[:], al[:], -1.0, al[:],
                                       op0=ALU.mult, op1=ALU.max)
        nc.vector.tensor_copy(outv[0:1, 0:1], ce_ps[0:1, 0:1])
        nc.vector.reduce_sum(outv[0:1, 1:2], osvdd[:], axis=AX.X)
        nc.vector.reduce_sum(outv[0:1, 2:3], csvdd[:], axis=AX.X)
        nc.vector.reduce_sum(outv[0:1, 3:4], al[:], axis=AX.X)
        nc.sync.dma_start(out=out_d[:], in_=outv[:])

    nc.compile()
    return nc


def _host_prep(inputs):
    f = np.float32
    xm8 = np.asarray(inputs["x_mid"], f).reshape(B, 512, 784).astype(F8)
    xd8 = np.asarray(inputs["x_deep"], f).reshape(B, 2048, 49).astype(F8)

    def T(w):
        return np.ascontiguousarray(np.asarray(w, f).T)

    def T8(w):
        return (T(w) * WSCALE).astype(F8)

    def ptile(w, kk):  # [K, O] -> [128, kk, O] with row k*128+p -> [p, k, :]
        K, O = w.shape
        return np.ascontiguousarray(w.reshape(kk, 128, O).transpose(1, 0, 2))

    M = np.asarray(inputs["w_shallow"], f).T @ np.asarray(inputs["sw1"], f).T

    center = np.asarray(inputs["center"], f)
    proto = np.asarray(inputs["proto"], f)
    catid = np.zeros((64, 128), f)
    catid[np.arange(64), np.arange(64)] = 1
    catid[np.arange(64), 64 + np.arange(64)] = 1
    ppad = np.zeros((4, 128), f)
    ppad[:, 64:] = -proto
    cpad = np.zeros((1, 128), f)
    cpad[0, 64:] = -center
    ones2 = np.zeros((98, 2), dtype=BF)
    ones2[0:49, 0] = 1
    ones2[49:98, 1] = 1
    o3s3 = np.concatenate([ptile(T8(inputs["ow3"]), 4),
                           ptile(T8(inputs["sw3"]), 4)], axis=2)
    blob128 = np.concatenate([T(inputs["tw1"]), T(inputs["cw1"])], axis=1)
    blob64 = np.concatenate(
        [T(inputs["tw2"]), T(inputs["cw2"]), T(inputs["qw1"]),
         T(inputs["qw2"]), T(proto), center.reshape(64, 1)], axis=1)

    shared = {
        "ow1T": ptile(T8(inputs["ow1"]), 16),
        "MT": ptile((M * WSCALE).astype(F8), 4),
        "ow2T": ptile(T8(inputs["ow2"]), 8),
        "sw2T": ptile(T8(inputs["sw2"]), 8),
        "o3s3": np.ascontiguousarray(o3s3),
        "blob128": np.ascontiguousarray(blob128),
        "blob64": np.ascontiguousarray(blob64),
        "cat_blob": catid,
        "ppad": ppad,
        "cpad": cpad,
        "id16": np.eye(16, dtype=f),
        "ones2": ones2,
        "onescol": np.ones((128, 1), dtype=BF),
    }
    in_maps = []
    for c in range(N_CORES):
        m = dict(shared)
        xc = xm8[c * BC:(c + 1) * BC]          # [16, 512, 784]
        xdc = xd8[c * BC:(c + 1) * BC]         # [16, 2048, 49]
        m["xmV"] = np.ascontiguousarray(xc[:, 0:128].transpose(1, 0, 2))
        m["xmA"] = np.ascontiguousarray(xc[:, 128:256].transpose(1, 0, 2))
        # xmP: [hw, (ct, b, c_lo)]
        m["xmP"] = np.ascontiguousarray(
            xc[:, 256:512].reshape(16, 2, 128, 784)
            .transpose(3, 1, 0, 2)).reshape(784, 4096)
        # xdV: b0-7, [d%128, b, d//128, hw]
        m["xdV"] = np.ascontiguousarray(
            xdc[0:8].reshape(8, 16, 128, 49).transpose(2, 0, 1, 3))
        # xdP: b8-15 packed 2 samples per partition set (even b upper, odd lower)
        hi = xdc[8:16]                          # [8, 2048, 49]
        ev = hi[0::2].transpose(2, 0, 1)        # [49, 4, 2048]
        od = hi[1::2].transpose(2, 0, 1)
        m["xdP"] = np.ascontiguousarray(
            np.concatenate([ev, od], axis=0)).reshape(98, 8192)
        in_maps.append(m)
    return in_maps


def _get_program():
    if "nc" not in _CACHE:
        _CACHE["nc"] = _build_program()
    return _CACHE["nc"]


def _combine(parts):
    tot = np.sum([np.asarray(p, np.float64).ravel() for p in parts], axis=0)
    return (tot / B).astype(np.float32).reshape(4, 1)


def _run(inputs, trace=False):
    from concourse.bass_utils import run_bass_kernel_spmd
    nc = _get_program()
    in_maps = _host_prep(inputs)
    kw = {}
    if trace:
        kw = dict(trace=True, trace_cores=list(range(N_CORES)))
    res = run_bass_kernel_spmd(nc, in_maps, list(range(N_CORES)), **kw)
    out = _combine([res.results[i]["out"] for i in range(N_CORES)])
    return out, res


def kernel(**inputs):
    out, _ = _run(inputs, trace=False)
    return out


def kernel_traced(**inputs):
    """Returns (output, exec_time_ns) using the NTFF profile (max over cores)."""
    out, res = _run(inputs, trace=True)
    return out, res.exec_time_ns


# revision 11
# speedup vs baseline: 1.2623x; 1.0571x over previous
"""DGAD net (vq_codebook) kernel v4.2 for 8x Trainium2 NeuronCores.

Contract: kernel(**inputs) takes FULL unsharded inputs, returns FULL [4,1]
fp32 output. Batch (128) sharded 16/core; weights replicated; final
all-reduce (sum/128) on host during unshard.

v4.2 vs v4.1 (71.5us):
  - All MLP evacs are single-op ACT Prelu(alpha=.01, scale=) — Prelu is in
    the natural_log_exp_and_others table set together with Copy/Exp/Ln/
    Square, so ONE table load at t0 covers every activation (no switches).
  - cat1/cat2 concat layers folded algebraically: tw1@[s;s-c] ==
    (tw1L+tw1R)@s + bias; cw1@[o;o-proto[cat]] == (cw1L+cw1R)@o + G@onehot.
    Kills catid/ppad/cpad tensors, 4 matmuls, 2 copies from the tail chain.
  - svdd distances via ACT Square(x + (-center)) single op (was sub+mult).
  - pnorm folded into the sim matmul as a 66th contraction row.
  - Small weights in one bf16 blob (f32 LDWEIGHTS was 300-700ns in tail).
  - Bulk DMAs merged to ~0.8-1.5MB pieces (HWDGE ring holds ~3 in flight;
    issue costs ~600ns each); MT/o3s3/blobs ride the slow ACT ring.
"""

import numpy as np
import ml_dtypes

N_CORES = 8
B = 128
BC = B // N_CORES  # 16 samples per core

BF = ml_dtypes.bfloat16
F8 = ml_dtypes.float8_e4m3
WSCALE = 256.0  # fp8 weights stored *256; 1/256 folded into consumer scales

_CACHE = {}


def _build_program():
    import concourse.bass as bass  # noqa: F401
    import concourse.mybir as mybir
    import concourse.tile as tile
    from concourse import bacc
    from contextlib import ExitStack

    dt = mybir.dt
    AF = mybir.ActivationFunctionType
    ALU = mybir.AluOpType
    AX = mybir.AxisListType
    f32, bf16, f8 = dt.float32, dt.bfloat16, dt.float8e4
    INV = 1.0 / WSCALE
    INV2 = INV * INV

    from concourse.hw_specs import get_activation_tables
    _act_set_id = list(get_activation_tables("gen3")).index("natural_log_exp_and_others")

    nc = bacc.Bacc("TRN2", target_bir_lowering=False, debug=False,
                   enable_asserts=True, num_devices=N_CORES)

    def din(name, shape, d):
        return nc.dram_tensor(name, shape, d, kind="ExternalInput").ap()

    xmV_d = din("xmV", [128, 16, 784], f8)    # ch 0-127, [c,b,hw] (DVE)
    xmA_d = din("xmA", [128, 16, 784], f8)    # ch 128-255, [c,b,hw] (ACT)
    xmP_d = din("xmP", [128, 6, 4096], f8)    # ch 256-511 hw<768, [hw%128, hw//128, (ct,b,c_lo)]
    xmP6_d = din("xmP6", [16, 4096], f8)      # hw 768-783 tail
    xdV_d = din("xdV", [128, 8, 16, 49], f8)  # b0-7, [d%128, b, d//128, hw] (DVE)
    xdP_d = din("xdP", [98, 8192], f8)        # b8-15 2-packed, [hw(+49*par), j*2048+d] (PE)
    ow1T_d = din("ow1T", [128, 16, 1024], f8)   # (k p) o -> p k o, *256
    MT_d = din("MT", [128, 4, 1024], f8)        # (wsh.T @ sw1.T)*256, pre-permuted
    ow2T_d = din("ow2T", [128, 8, 512], f8)
    sw2T_d = din("sw2T", [128, 8, 512], f8)
    o3s3_d = din("o3s3", [128, 4, 128], f8)     # cols 0:64 ow3T, 64:128 sw3T
    # blob64 cols: tw2|cw2|qw1|qw2|W1t|W2t|protoT|neg_center|bias_t1
    blob64_d = din("blob64", [64, 390], bf16)
    GT_d = din("GT", [4, 64], bf16)             # (-cw1R @ proto.T).T
    id16_d = din("id16", [16, 16], f32)
    protoF_d = din("protoF", [64, 4], f32)
    ones2_d = din("ones2", [98, 2], bf16)       # [:49]=[1,0], [49:]=[0,1]
    onescol_d = din("onescol", [128, 1], bf16)
    out_d = nc.dram_tensor("out", [1, 4], f32, kind="ExternalOutput").ap()

    with tile.TileContext(nc) as tc, ExitStack() as ctx:
        wp = ctx.enter_context(tc.tile_pool(name="wp", bufs=1))
        xp = ctx.enter_context(tc.tile_pool(name="xp", bufs=1))
        ap = ctx.enter_context(tc.tile_pool(name="ap", bufs=1))
        pmp = ctx.enter_context(tc.tile_pool(name="pmp", bufs=1, space="PSUM"))
        pdp = ctx.enter_context(tc.tile_pool(name="pdp", bufs=1, space="PSUM"))
        pbig = ctx.enter_context(tc.tile_pool(name="pbig", bufs=2, space="PSUM"))
        pt = ctx.enter_context(tc.tile_pool(name="pt", bufs=3, space="PSUM"))

        # ---------- tiles ----------
        ow1_t = wp.tile([128, 16, 1024], f8, tag="ow1")
        MT_t = wp.tile([128, 4, 1024], f8, tag="MT")
        ow2_t = wp.tile([128, 8, 512], f8, tag="ow2")
        sw2_t = wp.tile([128, 8, 512], f8, tag="sw2")
        o3s3_t = wp.tile([128, 4, 128], f8, tag="o3s3")
        blob64_t = wp.tile([64, 390], bf16, tag="blob64")
        GT_t = wp.tile([4, 64], bf16, tag="GT")
        id16_t = wp.tile([16, 16], f32, tag="id16")
        protoF_t = wp.tile([64, 4], f32, tag="protoF")
        ones2_t = wp.tile([98, 2], bf16, tag="ones2")
        onescol_t = wp.tile([128, 1], bf16, tag="onescol")

        xmV_t = xp.tile([128, 16, 784], f8, tag="xmV")
        xmA_t = xp.tile([128, 16, 784], f8, tag="xmA")
        xmP_t = xp.tile([128, 6, 4096], f8, tag="xmP")
        xmP6_t = xp.tile([16, 4096], f8, tag="xmP6")
        xdV_t = xp.tile([128, 8, 16, 49], f8, tag="xdV")
        xdP_t = xp.tile([98, 8192], f8, tag="xdP")

        tw2 = blob64_t[:, 0:64]
        cw2 = blob64_t[:, 64:128]
        qw1 = blob64_t[:, 128:192]
        qw2 = blob64_t[:, 192:256]
        W1t = blob64_t[:, 256:320]
        W2t = blob64_t[:, 320:384]
        protoT = blob64_t[:, 384:388]
        neg_cc = blob64_t[:, 388:389]
        bias_t1 = blob64_t[:, 389:390]

        # ---------- ACT: one table load covers Prelu/Copy/Exp/Ln/Square ----
        ldset = mybir.InstLoadActFuncSet(
            name=f"I-{nc.next_id()}", act_func_set_id=_act_set_id, ins=[], outs=[])
        ldset.engine = mybir.EngineType.Activation
        nc.scalar.add_instruction(ldset)

        # ---------- DMA issue: ACT ring (slow; tiny + late-needed only) -----
        for t_, d_ in ((ones2_t, ones2_d), (onescol_t, onescol_d),
                       (blob64_t, blob64_d), (o3s3_t, o3s3_d),
                       (id16_t, id16_d), (GT_t, GT_d),
                       (protoF_t, protoF_d)):
            nc.scalar.dma_start(out=t_[:], in_=d_)
        nc.scalar.dma_start(out=MT_t[:], in_=MT_d)

        # ---------- DMA issue: Sync ring, consumption-ordered ----------
        nc.sync.dma_start(out=xdV_t[:], in_=xdV_d)
        nc.sync.dma_start(out=xdP_t[:], in_=xdP_d)
        nc.sync.dma_start(out=ow1_t[:, 0:8, :], in_=ow1T_d[:, 0:8, :])
        nc.sync.dma_start(out=ow1_t[:, 8:16, :], in_=ow1T_d[:, 8:16, :])
        nc.sync.dma_start(out=ow2_t[:], in_=ow2T_d)
        nc.sync.dma_start(out=xmA_t[:, 0:8, :], in_=xmA_d[:, 0:8, :])
        nc.sync.dma_start(out=xmV_t[:, 0:8, :], in_=xmV_d[:, 0:8, :])
        nc.sync.dma_start(out=xmA_t[:, 8:16, :], in_=xmA_d[:, 8:16, :])
        nc.sync.dma_start(out=xmV_t[:, 8:16, :], in_=xmV_d[:, 8:16, :])
        nc.sync.dma_start(out=xmP_t[:, 0:3, :], in_=xmP_d[:, 0:3, :])
        nc.sync.dma_start(out=xmP_t[:, 3:6, :], in_=xmP_d[:, 3:6, :])
        nc.sync.dma_start(out=xmP6_t[:], in_=xmP6_d)
        nc.sync.dma_start(out=sw2_t[:], in_=sw2T_d)

        # ---------- gpsimd consts ----------
        ones64 = ap.tile([64, 1], f32, tag="ones64")
        nc.gpsimd.memset(ones64[:], 1.0)
        ones16 = ap.tile([16, 1], f32, tag="ones16")
        nc.gpsimd.memset(ones16[:], 1.0)
        rhs_sim = ap.tile([65, 4], f32, tag="rhs_sim")
        nc.gpsimd.memset(rhs_sim[64:65, :], 1.0)
        sim_lhs = ap.tile([65, 16], f32, tag="sim_lhs")
        ones1x16 = ap.tile([1, 16], f32, tag="ones1x16")
        nc.gpsimd.memset(ones1x16[:], 1.0)

        # ---------- sbuf activation tiles ----------
        pooled_v = ap.tile([128, 16], f32, tag="pooled_v")
        pooled_a = ap.tile([128, 16], f32, tag="pooled_a")
        pooled_dv = ap.tile([128, 8, 16], f32, tag="pooled_dv")
        scratch = ap.tile([128, 784], bf16, tag="scratch")
        xdb = ap.tile([128, 8, 2, 16], bf16, tag="xdb")   # b = 2j+s
        xmb = ap.tile([128, 16, 4], bf16, tag="xmb")
        y1o = ap.tile([128, 8, 16], bf16, tag="y1o")
        y2o = ap.tile([128, 4, 16], bf16, tag="y2o")
        origin = ap.tile([64, 16], bf16, tag="origin")
        q1 = ap.tile([64, 16], bf16, tag="q1")
        qf = ap.tile([64, 16], bf16, tag="qf")
        osq = ap.tile([64, 16], f32, tag="osq")
        osvdd = ap.tile([1, 16], f32, tag="osvdd")
        y1s = ap.tile([128, 8, 16], bf16, tag="y1s")
        y2s = ap.tile([128, 4, 16], bf16, tag="y2s")
        shallow = ap.tile([64, 16], bf16, tag="shallow")
        t1 = ap.tile([64, 16], bf16, tag="t1")
        t2 = ap.tile([64, 16], f32, tag="t2")
        sim_sb = ap.tile([16, 4], f32, tag="sim_sb")
        m16 = ap.tile([16, 1], f32, tag="m16")
        negm = ap.tile([16, 1], f32, tag="negm")
        onehotT = ap.tile([16, 4], f32, tag="onehotT")
        oh_sb = ap.tile([4, 16], bf16, tag="oh_sb")
        c1 = ap.tile([64, 16], bf16, tag="c1")
        cf = ap.tile([64, 16], bf16, tag="cf")
        csq = ap.tile([64, 16], f32, tag="csq")
        csvdd = ap.tile([1, 16], f32, tag="csvdd")
        al = ap.tile([1, 16], f32, tag="al")
        pT2 = ap.tile([64, 4], f32, tag="pT2")
        e_t = ap.tile([16, 4], f32, tag="e_t")
        s16 = ap.tile([16, 1], f32, tag="s16")
        ce_col = ap.tile([16, 1], f32, tag="ce_col")
        outv = ap.tile([1, 4], f32, tag="outv")

        pool_m = pmp.tile([128, 32], f32, tag="pool_m")       # col = ct*16+b
        pool_d = pdp.tile([128, 4, 16, 2], f32, tag="pool_d")  # [d%128, j, dc, s]

        def prelu(dst, src, scale=None, bias=None):
            kw = {}
            if scale is not None:
                kw["scale"] = scale
            if bias is not None:
                kw["bias"] = bias
            return nc.scalar.activation(dst, src, AF.Prelu, alpha=0.01, **kw)

        # ---------- DVE consts (dep: blob64) ----------
        nc.vector.tensor_tensor(pT2[:], protoF_t[:], protoF_t[:], op=ALU.mult)
        nc.vector.tensor_scalar(rhs_sim[0:64, :], protoF_t[:], -2.0, None,
                                op0=ALU.mult)

        # ---------- PE warm-up spin ----------
        warm_ps = pt.tile([128, 16], f32, tag="tail")
        for _ in range(12):
            nc.tensor.matmul(warm_ps[0:2, 0:2], ones2_t[:], ones2_t[:],
                             start=True, stop=True)

        # ---------- PE: x_deep b8-15 pool ----------
        for t in range(64):  # t = j*16 + dc
            nc.tensor.matmul(pool_d[:, t // 16, t % 16, :],
                             xdP_t[:, 128 * t:128 * t + 128],
                             ones2_t[:], start=True, stop=True)
        pn_ps = pt.tile([128, 16], f32, tag="tail")
        nc.tensor.matmul(pn_ps[0:1, 0:4], ones64[:], pT2[:], start=True, stop=True)

        # ---------- DVE: x_deep b0-7 pool + xdb ----------
        for hf in range(2):
            nc.vector.reduce_sum(pooled_dv[:, 4 * hf:4 * hf + 4, :],
                                 xdV_t[:, 4 * hf:4 * hf + 4, :, :], axis=AX.X)
        nc.vector.tensor_scalar(xdb[:, 0:4, :, :], pooled_dv[:], INV / 49.0,
                                None, op0=ALU.mult)
        for s in range(2):
            nc.vector.tensor_scalar(xdb[:, 4:8, s, :], pool_d[:, :, :, s],
                                    INV / 49.0, None, op0=ALU.mult)
        pnorm = ap.tile([1, 4], f32, tag="pnorm")
        nc.vector.tensor_copy(pnorm[:], pn_ps[0:1, 0:4])

        # ---------- PE: origin layer 1 (k-outer, one psum bank) ----------
        y1o_ps = pbig.tile([128, 8, 16], f32, tag="big")
        for k in range(16):
            for m in range(8):
                nc.tensor.matmul(y1o_ps[:, m, :],
                                 ow1_t[:, k, 128 * m:128 * m + 128],
                                 xdb[:, :, :, k],
                                 start=(k == 0), stop=(k == 15))

        # ---------- ACT: x_mid ch 128-255 pool (16 lines) ----------
        for b in range(16):
            nc.scalar.activation(scratch[:], xmA_t[:, b, :], AF.Copy,
                                 accum_out=pooled_a[:, b:b + 1])
        nc.scalar.mul(xmb[:, :, 1], pooled_a[:], INV / 784.0)

        # ---------- DVE: x_mid ch 0-127 pools ----------
        nc.vector.reduce_sum(pooled_v[:, 0:4], xmV_t[:, 0:4, :], axis=AX.X)
        prelu(y1o[:], y1o_ps[:])

        def pool_mm(h):
            """PE pool batch for xmP hw-tile h (32 MMs, chains over h)."""
            for t in range(32):
                if h < 6:
                    nc.tensor.matmul(pool_m[:, t:t + 1],
                                     xmP_t[:, h, 128 * t:128 * t + 128],
                                     onescol_t[:], start=(h == 0), stop=False)
                else:
                    nc.tensor.matmul(pool_m[:, t:t + 1],
                                     xmP6_t[:, 128 * t:128 * t + 128],
                                     onescol_t[0:16, :], start=False, stop=True)

        # ---------- PE: y2o + origin + q chain, interleaved with pools -----
        y2o_ps = pbig.tile([128, 4, 16], f32, tag="big")
        for k in range(8):
            for m in range(4):
                nc.tensor.matmul(y2o_ps[:, m, :],
                                 ow2_t[:, k, 128 * m:128 * m + 128],
                                 y1o[:, k, :], start=(k == 0), stop=(k == 7))
        pool_mm(0)

        nc.vector.reduce_sum(pooled_v[:, 4:8], xmV_t[:, 4:8, :], axis=AX.X)
        prelu(y2o[:], y2o_ps[:])

        origin_ps = pt.tile([128, 16], f32, tag="tail")
        for k in range(4):
            nc.tensor.matmul(origin_ps[0:64, :], o3s3_t[:, k, 0:64],
                             y2o[:, k, :], start=(k == 0), stop=(k == 3))
        pool_mm(1)
        prelu(origin[:], origin_ps[0:64, :], scale=INV2)

        q1_ps = pt.tile([128, 16], f32, tag="tail")
        nc.tensor.matmul(q1_ps[0:64, :], qw1, origin[:], start=True, stop=True)
        pool_mm(2)
        prelu(q1[:], q1_ps[0:64, :])
        q2_ps = pt.tile([128, 16], f32, tag="tail")
        nc.tensor.matmul(q2_ps[0:64, :], qw2, q1[:], start=True, stop=True)
        pool_mm(3)
        prelu(qf[:], q2_ps[0:64, :])
        nc.vector.reduce_sum(pooled_v[:, 8:12], xmV_t[:, 8:12, :], axis=AX.X)
        nc.scalar.activation(osq[:], qf[:], AF.Square, bias=neg_cc)
        osvdd_ps = pt.tile([128, 16], f32, tag="tail")
        nc.tensor.matmul(osvdd_ps[0:1, :], ones64[:], osq[:], start=True, stop=True)
        pool_mm(4)
        nc.vector.reduce_sum(pooled_v[:, 12:16], xmV_t[:, 12:16, :], axis=AX.X)
        nc.vector.tensor_copy(osvdd[:], osvdd_ps[0:1, :])
        nc.vector.tensor_scalar(xmb[:, :, 0], pooled_v[:], INV / 784.0, None,
                                op0=ALU.mult)
        pool_mm(5)
        pool_mm(6)

        # ---------- pool_m evac + M chain ----------
        for ct in range(2):
            nc.vector.tensor_scalar(xmb[:, :, 2 + ct],
                                    pool_m[:, 16 * ct:16 * ct + 16],
                                    INV / 784.0, None, op0=ALU.mult)
        y1s_ps = pbig.tile([128, 8, 16], f32, tag="big")
        for k in range(4):
            for m in range(8):
                nc.tensor.matmul(y1s_ps[:, m, :],
                                 MT_t[:, k, 128 * m:128 * m + 128],
                                 xmb[:, :, k], start=(k == 0), stop=(k == 3))
        prelu(y1s[:], y1s_ps[:])
        y2s_ps = pbig.tile([128, 4, 16], f32, tag="big")
        for k in range(8):
            for m in range(4):
                nc.tensor.matmul(y2s_ps[:, m, :],
                                 sw2_t[:, k, 128 * m:128 * m + 128],
                                 y1s[:, k, :], start=(k == 0), stop=(k == 7))
        prelu(y2s[:], y2s_ps[:])
        sh_ps = pt.tile([128, 16], f32, tag="tail")
        for k in range(4):
            nc.tensor.matmul(sh_ps[0:64, :], o3s3_t[:, k, 64:128],
                             y2s[:, k, :], start=(k == 0), stop=(k == 3))
        prelu(shallow[:], sh_ps[0:64, :], scale=INV2)

        # ---------- texture path (cat1 folded into W1t + bias_t1) ----------
        t1_ps = pt.tile([128, 16], f32, tag="tail")
        nc.tensor.matmul(t1_ps[0:64, :], W1t, shallow[:], start=True, stop=True)
        prelu(t1[:], t1_ps[0:64, :], bias=bias_t1)
        t2_ps = pt.tile([128, 16], f32, tag="tail")
        nc.tensor.matmul(t2_ps[0:64, :], tw2, t1[:], start=True, stop=True)
        prelu(sim_lhs[0:64, :], t2_ps[0:64, :])

        # ---------- sim + argmax + CE ----------
        nc.scalar.activation(t2[:], sim_lhs[0:64, :], AF.Square)
        tsq_ps = pt.tile([128, 16], f32, tag="tail")
        nc.tensor.matmul(tsq_ps[0:1, :], ones64[:], t2[:], start=True, stop=True)
        nc.vector.tensor_copy(sim_lhs[64:65, :], tsq_ps[0:1, :])
        sim_ps = pt.tile([128, 16], f32, tag="tail")
        nc.tensor.matmul(sim_ps[0:16, 0:4], sim_lhs[:], rhs_sim[:],
                         start=True, stop=False)
        nc.tensor.matmul(sim_ps[0:16, 0:4], ones1x16[:], pnorm[:],
                         start=False, stop=True)
        nc.vector.tensor_copy(sim_sb[:], sim_ps[0:16, 0:4])
        nc.vector.reduce_max(m16[:], sim_sb[:], axis=AX.X)
        nc.vector.reduce_max(negm[:], sim_sb[:], axis=AX.X, negate=True)
        nc.vector.tensor_scalar(onehotT[:], sim_sb[:], m16[:, 0:1], None,
                                op0=ALU.is_ge)
        nc.scalar.activation(e_t[:], sim_sb[:], AF.Exp, bias=negm[:, 0:1],
                             accum_out=s16[:])
        nc.scalar.activation(ce_col[:], s16[:], AF.Ln)
        oh_ps = pt.tile([128, 16], f32, tag="tail")
        nc.tensor.transpose(oh_ps[0:4, 0:16], onehotT[:], id16_t[:])
        nc.vector.tensor_copy(oh_sb[:], oh_ps[0:4, 0:16])

        # ---------- class feat chain (cat2 folded into W2t + GT) ----------
        c1_ps = pt.tile([128, 16], f32, tag="tail")
        nc.tensor.matmul(c1_ps[0:64, :], W2t, origin[:], start=True, stop=False)
        nc.tensor.matmul(c1_ps[0:64, :], GT_t[:], oh_sb[:], start=False, stop=True)
        ce_ps = pt.tile([128, 16], f32, tag="tail")
        nc.tensor.matmul(ce_ps[0:1, 0:1], ce_col[:], ones16[:],
                         start=True, stop=True)
        prelu(c1[:], c1_ps[0:64, :])
        cw2_ps = pt.tile([128, 16], f32, tag="tail")
        nc.tensor.matmul(cw2_ps[0:64, :], cw2, c1[:], start=True, stop=True)
        prelu(cf[:], cw2_ps[0:64, :])
        nc.scalar.activation(csq[:], cf[:], AF.Square, bias=neg_cc)
        csvdd_ps = pt.tile([128, 16], f32, tag="tail")
        nc.tensor.matmul(csvdd_ps[0:1, :], ones64[:], csq[:], start=True, stop=True)
        nc.vector.tensor_copy(csvdd[:], csvdd_ps[0:1, :])

        # ---------- align + output ----------
        nc.vector.tensor_tensor(al[:], osvdd[:], csvdd[:], op=ALU.subtract)
        nc.vector.scalar_tensor_tensor(al[:], al[:], -1.0, al[:],
                                       op0=ALU.mult, op1=ALU.max)
        nc.vector.tensor_copy(outv[0:1, 0:1], ce_ps[0:1, 0:1])
        nc.vector.reduce_sum(outv[0:1, 1:2], osvdd[:], axis=AX.X)
        nc.vector.reduce_sum(outv[0:1, 2:3], csvdd[:], axis=AX.X)
        nc.vector.reduce_sum(outv[0:1, 3:4], al[:], axis=AX.X)
        nc.sync.dma_start(out=out_d[:], in_=outv[:])

    nc.compile()
    return nc


def _host_prep(inputs):
    f = np.float32
    xm8 = np.asarray(inputs["x_mid"], f).reshape(B, 512, 784).astype(F8)
    xd8 = np.asarray(inputs["x_deep"], f).reshape(B, 2048, 49).astype(F8)

    def T(w):
        return np.ascontiguousarray(np.asarray(w, f).T)

    def T8(w):
        return (T(w) * WSCALE).astype(F8)

    def ptile(w, kk):  # [K, O] -> [128, kk, O] with row k*128+p -> [p, k, :]
        K, O = w.shape
        return np.ascontiguousarray(w.reshape(kk, 128, O).transpose(1, 0, 2))

    M = np.asarray(inputs["w_shallow"], f).T @ np.asarray(inputs["sw1"], f).T

    center = np.asarray(inputs["center"], f)
    proto = np.asarray(inputs["proto"], f)
    tw1 = np.asarray(inputs["tw1"], f)   # [64, 128]
    cw1 = np.asarray(inputs["cw1"], f)   # [64, 128]
    W1 = tw1[:, 0:64] + tw1[:, 64:128]   # t1 = W1 @ shallow + bias_t1
    bias_t1 = -(tw1[:, 64:128] @ center)  # [64]
    W2 = cw1[:, 0:64] + cw1[:, 64:128]   # c1 = W2 @ origin + G @ onehot
    G = -(cw1[:, 64:128] @ proto.T)      # [64, 4]
    ones2 = np.zeros((98, 2), dtype=BF)
    ones2[0:49, 0] = 1
    ones2[49:98, 1] = 1
    o3s3 = np.concatenate([ptile(T8(inputs["ow3"]), 4),
                           ptile(T8(inputs["sw3"]), 4)], axis=2)
    blob64 = np.concatenate(
        [T(inputs["tw2"]), T(inputs["cw2"]), T(inputs["qw1"]),
         T(inputs["qw2"]), T(W1), T(W2), T(proto),
         -center.reshape(64, 1), bias_t1.reshape(64, 1)],
        axis=1).astype(BF)

    shared = {
        "ow1T": ptile(T8(inputs["ow1"]), 16),
        "MT": ptile((M * WSCALE).astype(F8), 4),
        "ow2T": ptile(T8(inputs["ow2"]), 8),
        "sw2T": ptile(T8(inputs["sw2"]), 8),
        "o3s3": np.ascontiguousarray(o3s3),
        "blob64": np.ascontiguousarray(blob64),
        "GT": np.ascontiguousarray(T(G).astype(BF)),
        "id16": np.eye(16, dtype=f),
        "protoF": np.ascontiguousarray(T(proto)),
        "ones2": ones2,
        "onescol": np.ones((128, 1), dtype=BF),
    }
    in_maps = []
    for c in range(N_CORES):
        m = dict(shared)
        xc = xm8[c * BC:(c + 1) * BC]          # [16, 512, 784]
        xdc = xd8[c * BC:(c + 1) * BC]         # [16, 2048, 49]
        m["xmV"] = np.ascontiguousarray(xc[:, 0:128].transpose(1, 0, 2))
        m["xmA"] = np.ascontiguousarray(xc[:, 128:256].transpose(1, 0, 2))
        # xmP: [hw, (ct, b, c_lo)] -> [hw%128, hw//128, 4096] for hw<768
        xp_ = xc[:, 256:512].reshape(16, 2, 128, 784).transpose(3, 1, 0, 2) \
            .reshape(784, 4096)
        m["xmP"] = np.ascontiguousarray(
            xp_[0:768].reshape(6, 128, 4096).transpose(1, 0, 2))
        m["xmP6"] = np.ascontiguousarray(xp_[768:784])
        # xdV: b0-7, [d%128, b, d//128, hw]
        m["xdV"] = np.ascontiguousarray(
            xdc[0:8].reshape(8, 16, 128, 49).transpose(2, 0, 1, 3))
        # xdP: b8-15 packed 2 samples per partition set (even b upper, odd lower)
        hi = xdc[8:16]                          # [8, 2048, 49]
        ev = hi[0::2].transpose(2, 0, 1)        # [49, 4, 2048]
        od = hi[1::2].transpose(2, 0, 1)
        m["xdP"] = np.ascontiguousarray(
            np.concatenate([ev, od], axis=0)).reshape(98, 8192)
        in_maps.append(m)
    return in_maps


def _get_program():
    if "nc" not in _CACHE:
        _CACHE["nc"] = _build_program()
    return _CACHE["nc"]


def _combine(parts):
    tot = np.sum([np.asarray(p, np.float64).ravel() for p in parts], axis=0)
    return (tot / B).astype(np.float32).reshape(4, 1)


def _run(inputs, trace=False):
    from concourse.bass_utils import run_bass_kernel_spmd
    nc = _get_program()
    in_maps = _host_prep(inputs)
    kw = {}
    if trace:
        kw = dict(trace=True, trace_cores=list(range(N_CORES)))
    res = run_bass_kernel_spmd(nc, in_maps, list(range(N_CORES)), **kw)
    out = _combine([res.results[i]["out"] for i in range(N_CORES)])
    return out, res


def kernel(**inputs):
    out, _ = _run(inputs, trace=False)
    return out


def kernel_traced(**inputs):
    """Returns (output, exec_time_ns) using the NTFF profile (max over cores)."""
    out, res = _run(inputs, trace=True)
    return out, res.exec_time_ns


# revision 13
# speedup vs baseline: 1.3329x; 1.0560x over previous
"""DGAD net (vq_codebook) kernel v4.2 for 8x Trainium2 NeuronCores.

Contract: kernel(**inputs) takes FULL unsharded inputs, returns FULL [4,1]
fp32 output. Batch (128) sharded 16/core; weights replicated; final
all-reduce (sum/128) on host during unshard.

v4.2 vs v4.1 (71.5us):
  - All MLP evacs are single-op ACT Prelu(alpha=.01, scale=) — Prelu is in
    the natural_log_exp_and_others table set together with Copy/Exp/Ln/
    Square, so ONE table load at t0 covers every activation (no switches).
  - cat1/cat2 concat layers folded algebraically: tw1@[s;s-c] ==
    (tw1L+tw1R)@s + bias; cw1@[o;o-proto[cat]] == (cw1L+cw1R)@o + G@onehot.
    Kills catid/ppad/cpad tensors, 4 matmuls, 2 copies from the tail chain.
  - svdd distances via ACT Square(x + (-center)) single op (was sub+mult).
  - pnorm folded into the sim matmul as a 66th contraction row.
  - Small weights in one bf16 blob (f32 LDWEIGHTS was 300-700ns in tail).
  - Bulk DMAs merged to ~0.8-1.5MB pieces (HWDGE ring holds ~3 in flight;
    issue costs ~600ns each); MT/o3s3/blobs ride the slow ACT ring.
"""

import numpy as np
import ml_dtypes

N_CORES = 8
B = 128
BC = B // N_CORES  # 16 samples per core

BF = ml_dtypes.bfloat16
F8 = ml_dtypes.float8_e4m3
WSCALE = 256.0  # fp8 weights stored *256; 1/256 folded into consumer scales

_CACHE = {}


def _build_program():
    import concourse.bass as bass  # noqa: F401
    import concourse.mybir as mybir
    import concourse.tile as tile
    from concourse import bacc
    from contextlib import ExitStack

    dt = mybir.dt
    AF = mybir.ActivationFunctionType
    ALU = mybir.AluOpType
    AX = mybir.AxisListType
    f32, bf16, f8 = dt.float32, dt.bfloat16, dt.float8e4
    INV = 1.0 / WSCALE
    INV2 = INV * INV

    from concourse.hw_specs import get_activation_tables
    _act_set_id = list(get_activation_tables("gen3")).index("natural_log_exp_and_others")

    nc = bacc.Bacc("TRN2", target_bir_lowering=False, debug=False,
                   enable_asserts=True, num_devices=N_CORES)

    def din(name, shape, d):
        return nc.dram_tensor(name, shape, d, kind="ExternalInput").ap()

    xmV_d = din("xmV", [128, 16, 784], f8)    # ch 0-127, [c,b,hw] (DVE)
    xmA_d = din("xmA", [128, 16, 784], f8)    # ch 128-255, [c,b,hw] (ACT)
    xmP_d = din("xmP", [128, 6, 4096], f8)    # ch 256-511 hw<768, [hw%128, hw//128, (ct,b,c_lo)]
    xmP6_d = din("xmP6", [16, 4096], f8)      # hw 768-783 tail
    xdV_d = din("xdV", [128, 8, 16, 49], f8)  # b0-7, [d%128, b, d//128, hw] (DVE)
    xdP_d = din("xdP", [98, 8192], f8)        # b8-15 2-packed, [hw(+49*par), j*2048+d] (PE)
    ow1T_d = din("ow1T", [128, 16, 1024], f8)   # (k p) o -> p k o, *256
    MT_d = din("MT", [128, 4, 1024], f8)        # (wsh.T @ sw1.T)*256, pre-permuted
    ow2T_d = din("ow2T", [128, 8, 512], f8)
    sw2T_d = din("sw2T", [128, 8, 512], f8)
    o3s3_d = din("o3s3", [128, 4, 128], f8)     # cols 0:64 ow3T, 64:128 sw3T
    # blob64 cols: tw2|cw2|qw1|qw2|W1t|W2t|protoT|neg_center|bias_t1
    blob64_d = din("blob64", [64, 390], bf16)
    GT_d = din("GT", [4, 64], bf16)             # (-cw1R @ proto.T).T
    id16_d = din("id16", [16, 16], f32)
    protoF_d = din("protoF", [64, 4], f32)
    ones2_d = din("ones2", [98, 2], bf16)       # [:49]=[1,0], [49:]=[0,1]
    onescol_d = din("onescol", [128, 1], bf16)
    out_d = nc.dram_tensor("out", [1, 4], f32, kind="ExternalOutput").ap()

    with tile.TileContext(nc) as tc, ExitStack() as ctx:
        wp = ctx.enter_context(tc.tile_pool(name="wp", bufs=1))
        xp = ctx.enter_context(tc.tile_pool(name="xp", bufs=1))
        ap = ctx.enter_context(tc.tile_pool(name="ap", bufs=1))
        pmp = ctx.enter_context(tc.tile_pool(name="pmp", bufs=1, space="PSUM"))
        pdp = ctx.enter_context(tc.tile_pool(name="pdp", bufs=1, space="PSUM"))
        pbig = ctx.enter_context(tc.tile_pool(name="pbig", bufs=2, space="PSUM"))
        pt = ctx.enter_context(tc.tile_pool(name="pt", bufs=3, space="PSUM"))

        # ---------- tiles ----------
        ow1_t = wp.tile([128, 16, 1024], f8, tag="ow1")
        MT_t = wp.tile([128, 4, 1024], f8, tag="MT")
        ow2_t = wp.tile([128, 8, 512], f8, tag="ow2")
        sw2_t = wp.tile([128, 8, 512], f8, tag="sw2")
        o3s3_t = wp.tile([128, 4, 128], f8, tag="o3s3")
        blob64_t = wp.tile([64, 390], bf16, tag="blob64")
        GT_t = wp.tile([4, 64], bf16, tag="GT")
        id16_t = wp.tile([16, 16], f32, tag="id16")
        protoF_t = wp.tile([64, 4], f32, tag="protoF")
        ones2_t = wp.tile([98, 2], bf16, tag="ones2")
        onescol_t = wp.tile([128, 1], bf16, tag="onescol")

        xmV_t = xp.tile([128, 16, 784], f8, tag="xmV")
        xmA_t = xp.tile([128, 16, 784], f8, tag="xmA")
        xmP_t = xp.tile([128, 6, 4096], f8, tag="xmP")
        xmP6_t = xp.tile([16, 4096], f8, tag="xmP6")
        xdV_t = xp.tile([128, 8, 16, 49], f8, tag="xdV")
        xdP_t = xp.tile([98, 8192], f8, tag="xdP")

        tw2 = blob64_t[:, 0:64]
        cw2 = blob64_t[:, 64:128]
        qw1 = blob64_t[:, 128:192]
        qw2 = blob64_t[:, 192:256]
        W1t = blob64_t[:, 256:320]
        W2t = blob64_t[:, 320:384]
        protoT = blob64_t[:, 384:388]
        neg_cc = blob64_t[:, 388:389]
        bias_t1 = blob64_t[:, 389:390]

        # ---------- ACT: one table load covers Prelu/Copy/Exp/Ln/Square ----
        ldset = mybir.InstLoadActFuncSet(
            name=f"I-{nc.next_id()}", act_func_set_id=_act_set_id, ins=[], outs=[])
        ldset.engine = mybir.EngineType.Activation
        nc.scalar.add_instruction(ldset)

        # ---------- DMA issue: ACT ring (slow; tiny + late-needed only) -----
        for t_, d_ in ((ones2_t, ones2_d), (onescol_t, onescol_d),
                       (protoF_t, protoF_d), (blob64_t, blob64_d),
                       (o3s3_t, o3s3_d), (id16_t, id16_d), (GT_t, GT_d)):
            nc.scalar.dma_start(out=t_[:], in_=d_)
        nc.scalar.dma_start(out=MT_t[:], in_=MT_d)

        # ---------- DMA issue: Sync ring, consumption-ordered ----------
        nc.sync.dma_start(out=xdV_t[:], in_=xdV_d)
        nc.sync.dma_start(out=xdP_t[:], in_=xdP_d)
        nc.sync.dma_start(out=ow1_t[:, 0:8, :], in_=ow1T_d[:, 0:8, :])
        nc.sync.dma_start(out=ow1_t[:, 8:16, :], in_=ow1T_d[:, 8:16, :])
        nc.sync.dma_start(out=xmA_t[:, 0:8, :], in_=xmA_d[:, 0:8, :])
        nc.sync.dma_start(out=ow2_t[:], in_=ow2T_d)
        nc.sync.dma_start(out=xmP6_t[:], in_=xmP6_d)
        nc.sync.dma_start(out=xmV_t[:, 0:8, :], in_=xmV_d[:, 0:8, :])
        nc.sync.dma_start(out=xmA_t[:, 8:16, :], in_=xmA_d[:, 8:16, :])
        nc.sync.dma_start(out=xmP_t[:, 0:2, :], in_=xmP_d[:, 0:2, :])
        nc.sync.dma_start(out=xmV_t[:, 8:16, :], in_=xmV_d[:, 8:16, :])
        nc.sync.dma_start(out=xmP_t[:, 2:4, :], in_=xmP_d[:, 2:4, :])
        nc.sync.dma_start(out=xmP_t[:, 4:5, :], in_=xmP_d[:, 4:5, :])
        nc.sync.dma_start(out=xmP_t[:, 5:6, :], in_=xmP_d[:, 5:6, :])
        nc.sync.dma_start(out=sw2_t[:], in_=sw2T_d)

        # ---------- gpsimd consts ----------
        ones64 = ap.tile([64, 1], f32, tag="ones64")
        nc.gpsimd.memset(ones64[:], 1.0)
        ones16 = ap.tile([16, 1], f32, tag="ones16")
        nc.gpsimd.memset(ones16[:], 1.0)
        rhs_sim = ap.tile([65, 4], f32, tag="rhs_sim")
        nc.gpsimd.memset(rhs_sim[64:65, :], 1.0)
        sim_lhs = ap.tile([65, 16], f32, tag="sim_lhs")
        ones1x16 = ap.tile([1, 16], f32, tag="ones1x16")
        nc.gpsimd.memset(ones1x16[:], 1.0)

        # ---------- sbuf activation tiles ----------
        pooled_v = ap.tile([128, 16], f32, tag="pooled_v")
        pooled_a = ap.tile([128, 16], f32, tag="pooled_a")
        pooled_dv = ap.tile([128, 8, 16], f32, tag="pooled_dv")
        scratch = ap.tile([128, 784], bf16, tag="scratch")
        xdb = ap.tile([128, 8, 2, 16], bf16, tag="xdb")   # b = 2j+s
        xmb = ap.tile([128, 16, 4], bf16, tag="xmb")
        y1o = ap.tile([128, 8, 16], bf16, tag="y1o")
        y2o = ap.tile([128, 4, 16], bf16, tag="y2o")
        origin = ap.tile([64, 16], bf16, tag="origin")
        q1 = ap.tile([64, 16], bf16, tag="q1")
        qf = ap.tile([64, 16], bf16, tag="qf")
        osq = ap.tile([64, 16], f32, tag="osq")
        osvdd = ap.tile([1, 16], f32, tag="osvdd")
        y1s = ap.tile([128, 8, 16], bf16, tag="y1s")
        y2s = ap.tile([128, 4, 16], bf16, tag="y2s")
        shallow = ap.tile([64, 16], bf16, tag="shallow")
        t1 = ap.tile([64, 16], bf16, tag="t1")
        t2 = ap.tile([64, 16], f32, tag="t2")
        sim_sb = ap.tile([16, 4], f32, tag="sim_sb")
        m16 = ap.tile([16, 1], f32, tag="m16")
        negm = ap.tile([16, 1], f32, tag="negm")
        onehotT = ap.tile([16, 4], f32, tag="onehotT")
        oh_sb = ap.tile([4, 16], bf16, tag="oh_sb")
        c1 = ap.tile([64, 16], bf16, tag="c1")
        cf = ap.tile([64, 16], bf16, tag="cf")
        csq = ap.tile([64, 16], f32, tag="csq")
        csvdd = ap.tile([1, 16], f32, tag="csvdd")
        al = ap.tile([1, 16], f32, tag="al")
        pT2 = ap.tile([64, 4], f32, tag="pT2")
        e_t = ap.tile([16, 4], f32, tag="e_t")
        s16 = ap.tile([16, 1], f32, tag="s16")
        ce_col = ap.tile([16, 1], f32, tag="ce_col")
        outv = ap.tile([1, 4], f32, tag="outv")

        pool_m = pmp.tile([128, 32], f32, tag="pool_m")       # col = ct*16+b
        pool_d = pdp.tile([128, 4, 16, 2], f32, tag="pool_d")  # [d%128, j, dc, s]

        def prelu(dst, src, scale=None, bias=None):
            kw = {}
            if scale is not None:
                kw["scale"] = scale
            if bias is not None:
                kw["bias"] = bias
            return nc.scalar.activation(dst, src, AF.Prelu, alpha=0.01, **kw)

        # ---------- PE warm-up spin ----------
        warm_ps = pt.tile([128, 16], f32, tag="tail")
        for _ in range(12):
            nc.tensor.matmul(warm_ps[0:2, 0:2], ones2_t[:], ones2_t[:],
                             start=True, stop=True)

        # ---------- PE: x_deep b8-15 pool ----------
        for t in range(64):  # t = j*16 + dc
            nc.tensor.matmul(pool_d[:, t // 16, t % 16, :],
                             xdP_t[:, 128 * t:128 * t + 128],
                             ones2_t[:], start=True, stop=True)
        # ---------- DVE: x_deep b0-7 pool + xdb ----------
        for hf in range(2):
            nc.vector.reduce_sum(pooled_dv[:, 4 * hf:4 * hf + 4, :],
                                 xdV_t[:, 4 * hf:4 * hf + 4, :, :], axis=AX.X)
        nc.vector.tensor_scalar(xdb[:, 0:4, :, :], pooled_dv[:], INV / 49.0,
                                None, op0=ALU.mult)
        for s in range(2):
            nc.vector.tensor_scalar(xdb[:, 4:8, s, :], pool_d[:, :, :, s],
                                    INV / 49.0, None, op0=ALU.mult)
        nc.vector.tensor_tensor(pT2[:], protoF_t[:], protoF_t[:], op=ALU.mult)
        nc.vector.tensor_scalar(rhs_sim[0:64, :], protoF_t[:], -2.0, None,
                                op0=ALU.mult)
        pn_ps = pt.tile([128, 16], f32, tag="tail")
        nc.tensor.matmul(pn_ps[0:1, 0:4], ones64[:], pT2[:], start=True, stop=True)
        pnorm = ap.tile([1, 4], f32, tag="pnorm")
        nc.vector.tensor_copy(pnorm[:], pn_ps[0:1, 0:4])

        # ---------- PE: origin layer 1 (k-outer, one psum bank) ----------
        y1o_ps = pbig.tile([128, 8, 16], f32, tag="big")
        for k in range(16):
            for m in range(8):
                nc.tensor.matmul(y1o_ps[:, m, :],
                                 ow1_t[:, k, 128 * m:128 * m + 128],
                                 xdb[:, :, :, k],
                                 start=(k == 0), stop=(k == 15))

        # ---------- ACT: x_mid ch 128-255 pool (16 lines) ----------
        for b in range(16):
            nc.scalar.activation(scratch[:], xmA_t[:, b, :], AF.Copy,
                                 accum_out=pooled_a[:, b:b + 1])
        nc.scalar.mul(xmb[:, :, 1], pooled_a[:], INV / 784.0)

        # ---------- DVE: x_mid ch 0-127 pools ----------
        nc.vector.reduce_sum(pooled_v[:, 0:4], xmV_t[:, 0:4, :], axis=AX.X)
        prelu(y1o[:], y1o_ps[:])

        def pool_mm(h):
            """PE pool batch for xmP hw-tile h (32 MMs, chains over h)."""
            for t in range(32):
                if h < 6:
                    nc.tensor.matmul(pool_m[:, t:t + 1],
                                     xmP_t[:, h, 128 * t:128 * t + 128],
                                     onescol_t[:], start=(h == 0), stop=False)
                else:
                    nc.tensor.matmul(pool_m[:, t:t + 1],
                                     xmP6_t[:, 128 * t:128 * t + 128],
                                     onescol_t[0:16, :], start=False, stop=True)

        # ---------- PE: y2o + origin + q chain, interleaved with pools -----
        y2o_ps = pbig.tile([128, 4, 16], f32, tag="big")
        for k in range(8):
            for m in range(4):
                nc.tensor.matmul(y2o_ps[:, m, :],
                                 ow2_t[:, k, 128 * m:128 * m + 128],
                                 y1o[:, k, :], start=(k == 0), stop=(k == 7))
        pool_mm(0)

        nc.vector.reduce_sum(pooled_v[:, 4:8], xmV_t[:, 4:8, :], axis=AX.X)
        prelu(y2o[:], y2o_ps[:])

        origin_ps = pt.tile([128, 16], f32, tag="tail")
        for k in range(4):
            nc.tensor.matmul(origin_ps[0:64, :], o3s3_t[:, k, 0:64],
                             y2o[:, k, :], start=(k == 0), stop=(k == 3))
        pool_mm(1)
        prelu(origin[:], origin_ps[0:64, :], scale=INV2)

        q1_ps = pt.tile([128, 16], f32, tag="tail")
        nc.tensor.matmul(q1_ps[0:64, :], qw1, origin[:], start=True, stop=True)
        pool_mm(2)
        prelu(q1[:], q1_ps[0:64, :])
        q2_ps = pt.tile([128, 16], f32, tag="tail")
        nc.tensor.matmul(q2_ps[0:64, :], qw2, q1[:], start=True, stop=True)
        pool_mm(3)
        prelu(qf[:], q2_ps[0:64, :])
        nc.vector.reduce_sum(pooled_v[:, 8:12], xmV_t[:, 8:12, :], axis=AX.X)
        nc.scalar.activation(osq[:], qf[:], AF.Square, bias=neg_cc)
        osvdd_ps = pt.tile([128, 16], f32, tag="tail")
        nc.tensor.matmul(osvdd_ps[0:1, :], ones64[:], osq[:], start=True, stop=True)
        pool_mm(4)
        nc.vector.reduce_sum(pooled_v[:, 12:16], xmV_t[:, 12:16, :], axis=AX.X)
        nc.vector.tensor_copy(osvdd[:], osvdd_ps[0:1, :])
        nc.vector.tensor_scalar(xmb[:, :, 0], pooled_v[:], INV / 784.0, None,
                                op0=ALU.mult)
        y1s_ps = pbig.tile([128, 8, 16], f32, tag="big")

        def m_layer(k, start, stop):
            for m in range(8):
                nc.tensor.matmul(y1s_ps[:, m, :],
                                 MT_t[:, k, 128 * m:128 * m + 128],
                                 xmb[:, :, k], start=start, stop=stop)

        m_layer(0, True, False)
        m_layer(1, False, False)
        pool_mm(5)
        pool_mm(6)

        # ---------- pool_m evac + M chain tail ----------
        for ct in range(2):
            nc.vector.tensor_scalar(xmb[:, :, 2 + ct],
                                    pool_m[:, 16 * ct:16 * ct + 16],
                                    INV / 784.0, None, op0=ALU.mult)
        m_layer(2, False, False)
        m_layer(3, False, True)
        prelu(y1s[:, 0:4, :], y1s_ps[:, 0:4, :])
        prelu(y1s[:, 4:8, :], y1s_ps[:, 4:8, :])
        y2s_ps = pbig.tile([128, 4, 16], f32, tag="big")
        for k in range(8):
            for m in range(4):
                nc.tensor.matmul(y2s_ps[:, m, :],
                                 sw2_t[:, k, 128 * m:128 * m + 128],
                                 y1s[:, k, :], start=(k == 0), stop=(k == 7))
        prelu(y2s[:], y2s_ps[:])
        sh_ps = pt.tile([128, 16], f32, tag="tail")
        for k in range(4):
            nc.tensor.matmul(sh_ps[0:64, :], o3s3_t[:, k, 64:128],
                             y2s[:, k, :], start=(k == 0), stop=(k == 3))
        prelu(shallow[:], sh_ps[0:64, :], scale=INV2)

        # ---------- texture path (cat1 folded into W1t + bias_t1) ----------
        t1_ps = pt.tile([128, 16], f32, tag="tail")
        nc.tensor.matmul(t1_ps[0:64, :], W1t, shallow[:], start=True, stop=True)
        prelu(t1[:], t1_ps[0:64, :], bias=bias_t1)
        t2_ps = pt.tile([128, 16], f32, tag="tail")
        nc.tensor.matmul(t2_ps[0:64, :], tw2, t1[:], start=True, stop=True)
        prelu(sim_lhs[0:64, :], t2_ps[0:64, :])

        # ---------- sim + argmax + CE ----------
        nc.scalar.activation(t2[:], sim_lhs[0:64, :], AF.Square)
        tsq_ps = pt.tile([128, 16], f32, tag="tail")
        nc.tensor.matmul(tsq_ps[0:1, :], ones64[:], t2[:], start=True, stop=True)
        nc.vector.tensor_copy(sim_lhs[64:65, :], tsq_ps[0:1, :])
        sim_ps = pt.tile([128, 16], f32, tag="tail")
        nc.tensor.matmul(sim_ps[0:16, 0:4], sim_lhs[:], rhs_sim[:],
                         start=True, stop=False)
        nc.tensor.matmul(sim_ps[0:16, 0:4], ones1x16[:], pnorm[:],
                         start=False, stop=True)
        nc.vector.tensor_copy(sim_sb[:], sim_ps[0:16, 0:4])
        nc.vector.reduce_max(m16[:], sim_sb[:], axis=AX.X)
        nc.vector.reduce_max(negm[:], sim_sb[:], axis=AX.X, negate=True)
        nc.vector.tensor_scalar(onehotT[:], sim_sb[:], m16[:, 0:1], None,
                                op0=ALU.is_ge)
        nc.scalar.activation(e_t[:], sim_sb[:], AF.Exp, bias=negm[:, 0:1],
                             accum_out=s16[:])
        nc.scalar.activation(ce_col[:], s16[:], AF.Ln)
        oh_ps = pt.tile([128, 16], f32, tag="tail")
        nc.tensor.transpose(oh_ps[0:4, 0:16], onehotT[:], id16_t[:])
        nc.vector.tensor_copy(oh_sb[:], oh_ps[0:4, 0:16])

        # ---------- class feat chain (cat2 folded into W2t + GT) ----------
        c1_ps = pt.tile([128, 16], f32, tag="tail")
        nc.tensor.matmul(c1_ps[0:64, :], W2t, origin[:], start=True, stop=False)
        nc.tensor.matmul(c1_ps[0:64, :], GT_t[:], oh_sb[:], start=False, stop=True)
        ce_ps = pt.tile([128, 16], f32, tag="tail")
        nc.tensor.matmul(ce_ps[0:1, 0:1], ce_col[:], ones16[:],
                         start=True, stop=True)
        prelu(c1[:], c1_ps[0:64, :])
        cw2_ps = pt.tile([128, 16], f32, tag="tail")
        nc.tensor.matmul(cw2_ps[0:64, :], cw2, c1[:], start=True, stop=True)
        prelu(cf[:], cw2_ps[0:64, :])
        nc.scalar.activation(csq[:], cf[:], AF.Square, bias=neg_cc)
        csvdd_ps = pt.tile([128, 16], f32, tag="tail")
        nc.tensor.matmul(csvdd_ps[0:1, :], ones64[:], csq[:], start=True, stop=True)
        # ---------- align + output ----------
        nc.vector.tensor_tensor(al[:], osvdd[:], csvdd_ps[0:1, :], op=ALU.subtract)
        nc.vector.scalar_tensor_tensor(al[:], al[:], -1.0, al[:],
                                       op0=ALU.mult, op1=ALU.max)
        nc.vector.tensor_copy(outv[0:1, 0:1], ce_ps[0:1, 0:1])
        nc.vector.reduce_sum(outv[0:1, 1:2], osvdd[:], axis=AX.X)
        nc.vector.reduce_sum(outv[0:1, 2:3], csvdd_ps[0:1, :], axis=AX.X)
        nc.vector.reduce_sum(outv[0:1, 3:4], al[:], axis=AX.X)
        nc.sync.dma_start(out=out_d[:], in_=outv[:])

    nc.compile()
    return nc


def _host_prep(inputs):
    f = np.float32
    xm8 = np.asarray(inputs["x_mid"], f).reshape(B, 512, 784).astype(F8)
    xd8 = np.asarray(inputs["x_deep"], f).reshape(B, 2048, 49).astype(F8)

    def T(w):
        return np.ascontiguousarray(np.asarray(w, f).T)

    def T8(w):
        return (T(w) * WSCALE).astype(F8)

    def ptile(w, kk):  # [K, O] -> [128, kk, O] with row k*128+p -> [p, k, :]
        K, O = w.shape
        return np.ascontiguousarray(w.reshape(kk, 128, O).transpose(1, 0, 2))

    M = np.asarray(inputs["w_shallow"], f).T @ np.asarray(inputs["sw1"], f).T

    center = np.asarray(inputs["center"], f)
    proto = np.asarray(inputs["proto"], f)
    tw1 = np.asarray(inputs["tw1"], f)   # [64, 128]
    cw1 = np.asarray(inputs["cw1"], f)   # [64, 128]
    W1 = tw1[:, 0:64] + tw1[:, 64:128]   # t1 = W1 @ shallow + bias_t1
    bias_t1 = -(tw1[:, 64:128] @ center)  # [64]
    W2 = cw1[:, 0:64] + cw1[:, 64:128]   # c1 = W2 @ origin + G @ onehot
    G = -(cw1[:, 64:128] @ proto.T)      # [64, 4]
    ones2 = np.zeros((98, 2), dtype=BF)
    ones2[0:49, 0] = 1
    ones2[49:98, 1] = 1
    o3s3 = np.concatenate([ptile(T8(inputs["ow3"]), 4),
                           ptile(T8(inputs["sw3"]), 4)], axis=2)
    blob64 = np.concatenate(
        [T(inputs["tw2"]), T(inputs["cw2"]), T(inputs["qw1"]),
         T(inputs["qw2"]), T(W1), T(W2), T(proto),
         -center.reshape(64, 1), bias_t1.reshape(64, 1)],
        axis=1).astype(BF)

    shared = {
        "ow1T": ptile(T8(inputs["ow1"]), 16),
        "MT": ptile((M * WSCALE).astype(F8), 4),
        "ow2T": ptile(T8(inputs["ow2"]), 8),
        "sw2T": ptile(T8(inputs["sw2"]), 8),
        "o3s3": np.ascontiguousarray(o3s3),
        "blob64": np.ascontiguousarray(blob64),
        "GT": np.ascontiguousarray(T(G).astype(BF)),
        "id16": np.eye(16, dtype=f),
        "protoF": np.ascontiguousarray(T(proto)),
        "ones2": ones2,
        "onescol": np.ones((128, 1), dtype=BF),
    }
    in_maps = []
    for c in range(N_CORES):
        m = dict(shared)
        xc = xm8[c * BC:(c + 1) * BC]          # [16, 512, 784]
        xdc = xd8[c * BC:(c + 1) * BC]         # [16, 2048, 49]
        m["xmV"] = np.ascontiguousarray(xc[:, 0:128].transpose(1, 0, 2))
        m["xmA"] = np.ascontiguousarray(xc[:, 128:256].transpose(1, 0, 2))
        # xmP: [hw, (ct, b, c_lo)] -> [hw%128, hw//128, 4096] for hw<768
        xp_ = xc[:, 256:512].reshape(16, 2, 128, 784).transpose(3, 1, 0, 2) \
            .reshape(784, 4096)
        m["xmP"] = np.ascontiguousarray(
            xp_[0:768].reshape(6, 128, 4096).transpose(1, 0, 2))
        m["xmP6"] = np.ascontiguousarray(xp_[768:784])
        # xdV: b0-7, [d%128, b, d//128, hw]
        m["xdV"] = np.ascontiguousarray(
            xdc[0:8].reshape(8, 16, 128, 49).transpose(2, 0, 1, 3))
        # xdP: b8-15 packed 2 samples per partition set (even b upper, odd lower)
        hi = xdc[8:16]                          # [8, 2048, 49]
        ev = hi[0::2].transpose(2, 0, 1)        # [49, 4, 2048]
        od = hi[1::2].transpose(2, 0, 1)
        m["xdP"] = np.ascontiguousarray(
            np.concatenate([ev, od], axis=0)).reshape(98, 8192)
        in_maps.append(m)
    return in_maps


def _get_program():
    if "nc" not in _CACHE:
        _CACHE["nc"] = _build_program()
    return _CACHE["nc"]


def _combine(parts):
    tot = np.sum([np.asarray(p, np.float64).ravel() for p in parts], axis=0)
    return (tot / B).astype(np.float32).reshape(4, 1)


def _run(inputs, trace=False):
    from concourse.bass_utils import run_bass_kernel_spmd
    nc = _get_program()
    in_maps = _host_prep(inputs)
    kw = {}
    if trace:
        kw = dict(trace=True, trace_cores=list(range(N_CORES)))
    res = run_bass_kernel_spmd(nc, in_maps, list(range(N_CORES)), **kw)
    out = _combine([res.results[i]["out"] for i in range(N_CORES)])
    return out, res


def kernel(**inputs):
    out, _ = _run(inputs, trace=False)
    return out


def kernel_traced(**inputs):
    """Returns (output, exec_time_ns) using the NTFF profile (max over cores)."""
    out, res = _run(inputs, trace=True)
    return out, res.exec_time_ns
